# revision 25
# baseline (speedup 1.0000x reference)
"""Trainium2 Bass kernel for BatchedGNNModel (4-layer GCN over 3-rod chain graph).

Contract: kernel(**inputs) takes FULL unsharded inputs (as produced by
setup_inputs) and returns the FULL (64, 768, 3) float32 output.

Sharding: pure data parallel over batch — 8 items per NeuronCore on 8 cores,
identical SPMD program, weights replicated (marshaled on host).

v4 fast-path algorithm (zero biases, expected adjacency structure):
  A_norm = D·M·D with D = diag(d), d = deg^-1/2, M = tridiagonal-support ones
  + ~10 coefficient-1 sparse corrections. One application of M is
  S = tri_shift(U) + ents(U). d² is 1/3 everywhere except 8 columns
  ({0,255,511,767}: 1/2, {100,200,256,512}: 1/5), so every d²⊙ plane multiply
  is a tensor_scalar ×(1/3) plus 4 tiny strided column-fix multiplies — no
  d² plane in SBUF or DMA at all.

  Folded chain (relu is positively homogeneous, feature ops commute with
  node-diagonal scales):
    Gp = d²⊙(M (d⊙x))          [L1: DVE tri+ents+scale on packed x]
    h1 = relu(Gp @ W1ᵀ)        [f1: PE per-item 6-contract matmuls, ACT/DVE evac]
    u2 = h1 @ W2ᵀ              [f2: PE 128-contract matmuls per item pair]
    s2 = relu(M u2)            [agg2: DVE tri+ents+relu per pair]
    u3 = d²⊙(s2 @ WC)          [f4: PE item-packed 32-col matmuls, WC = W3ᵀW4ᵀ]
    out = d ⊙ M (d²⊙(M u3))    [tail per 4-item group: S3, m4, S4; d⊙ on host]

  Layout: items packed 4 per group at partition stride 32, features 0:6 of
  each 32-band; 2 groups as column blocks of 768. All activations bf16.
  Per-item software pipeline: f1(i)→f2(pair)→agg2(pair)→f4(i), with group 0's
  whole tail + output DMA issued mid-kernel so only group 1's tail is exposed.
  Input DMAs are row-sparse (only the 6 used partitions per band) and split
  across the two HWDGE queues (sync: x, scalar: weights) so descriptor
  writing parallelizes and first compute starts ~3us earlier.

Fallback path (nonzero biases or unexpected adjacency/d²): v1 dense program.

This image's walrus accepts only one sync-wait slot per instruction, so a
post-pass splits Tile's multi-wait instructions into single-wait NoOps.
"""

import os
import sys

import numpy as np

sys.path.insert(0, "/opt/trn_rl_repo")

import ml_dtypes
import concourse.bass as bass
import concourse.mybir as mybir
import concourse.tile as _tile_mod
from concourse.tile import TileContext
from concourse.vector_clock import ScopedClock
from concourse.bass_utils import run_bass_kernel_spmd


def _patched_drain_and_barrier(self, tick_clock, wait_clock):
    """The nix walrus in this image only supports one sync-wait slot on a
    Drain; Tile's kernel-tail drain carries one wait per ticked semaphore.
    Split the extra waits onto single-wait nops on the same (sync) engine —
    program order makes this equivalent before the all-engine barrier."""
    drain_inst = self.nc.sync.drain()
    wait_clock.add_sem_waits(
        drain_inst.ins, ScopedClock({None: tick_clock.global_clock}))
    waits = list(drain_inst.ins.sync_info.on_wait)
    if len(waits) > 1:
        import bass_rust
        drain_inst.ins.sync_info.on_wait = [waits[0]]
        for w in waits[1:]:
            nop = self.nc.sync.nop(nofuse=True)
            si = nop.ins.sync_info
            if si is None:
                nop.ins.sync_info = bass_rust.SyncInfo(on_wait=[w], on_update=[])
            else:
                si.on_wait = [w]
    self.nc.all_engine_barrier()
    assert self.sems is not None
    popped = self.nc._tile_sem_poison_stack.pop()
    assert popped is self._sem_poison
    self.nc.clear_and_free_semaphores(list(self.sems.allocated().values()))
    self.nc.all_engine_barrier()


_tile_mod.TileContext._drain_and_barrier = _patched_drain_and_barrier


def _split_multi_waits(nc):
    """This image's walrus supports a single sync-wait slot per instruction.
    Hoist all-but-one wait of any multi-wait instruction onto single-wait
    NoOps on the same engine, placed immediately before it (same per-engine
    program order => equivalent synchronization)."""
    for f in nc.m.functions:
        for bb in f.blocks:
            insts = list(bb.instructions)
            if not any(ins.sync_info and len(ins.sync_info.on_wait) > 1
                       for ins in insts):
                continue
            new = []
            for ins in insts:
                si = ins.sync_info
                if si is not None and len(si.on_wait) > 1:
                    waits = list(si.on_wait)
                    for w in waits[:-1]:
                        new.append(mybir.InstNoOp(
                            name=nc.get_next_instruction_name(),
                            sync_info=mybir.SyncInfo(on_wait=[w], on_update=[]),
                            bass_nofuse=True,
                            engine=ins.engine,
                        ))
                    si.on_wait = [waits[-1]]
                new.append(ins)
            bb.instructions = new


def _ensure_ntff_hook():
    """The agent image's antenv lacks axon_hooks; bass_utils imports it when
    trace=True. Install a shim and, if possible, the real ctypes profiler."""
    import types
    try:
        import antenv.axon_hooks  # noqa: F401
        return
    except Exception:
        pass
    try:
        import antenv
        mod = types.ModuleType("antenv.axon_hooks")
        state = {"h": None}
        mod.set_axon_ntff_profile_hook = lambda h: state.__setitem__("h", h)
        mod.get_axon_ntff_profile_hook = lambda: state["h"]
        sys.modules["antenv.axon_hooks"] = mod
        antenv.axon_hooks = mod
        try:
            from trn_agent_boot.trn_boot import _ntff_profile_via_ctypes
            mod.set_axon_ntff_profile_hook(
                _ntff_profile_via_ctypes("/opt/axon/libaxon_pjrt.so"))
        except Exception:
            pass
    except Exception:
        pass


_ensure_ntff_hook()

F32 = mybir.dt.float32
BF16 = mybir.dt.bfloat16
RELU = mybir.ActivationFunctionType.Relu
ADD = mybir.AluOpType.add
SUB = mybir.AluOpType.subtract
MULT = mybir.AluOpType.mult
MAX = mybir.AluOpType.max

B = 64
NV = 256
N = 3 * NV  # 768
NCORES = 8
IPC = B // NCORES  # 8 items per core

ONE_THIRD = float(np.float32(1.0) / np.float32(3.0))

LAST_RUN_INFO = {}

# Sparse corrections for one M application, coefficient-1 form, order-safe:
# (dst_col, 'S'|'U', src_col, op). S reads must precede writes to their col.
ENT_OPS = [
    (256, 'S', 100, ADD), (512, 'S', 200, ADD),
    (256, 'U', 255, SUB), (512, 'U', 511, SUB),
    (100, 'U', 256, ADD), (100, 'U', 257, ADD),
    (200, 'U', 512, ADD), (200, 'U', 513, ADD),
    (255, 'U', 256, SUB), (511, 'U', 512, SUB),
]

# d² = 1/3 everywhere except: ×3/2 at {0,255,511,767}, ×3/5 at {100,200,256,512}
FIX_GROUPS = [
    ((0, 1, 1), 1.5),
    ((255, 768, 256), 1.5),
    ((100, 201, 100), 0.6),
    ((256, 513, 256), 0.6),
]


def _np_tri_shift(U):
    S = U.copy()
    S[..., 1:, :] += U[..., :-1, :]
    S[..., :-1, :] += U[..., 1:, :]
    return S


def _np_ents(S, U):
    for (j, kind, k, op) in ENT_OPS:
        src = (S if kind == 'S' else U)[..., k, :].copy()
        if op is ADD:
            S[..., j, :] += src
        else:
            S[..., j, :] -= src
    return S


def _structure_matches(A_norm, d):
    """Does d ⊙ (tri+ents)(d ⊙ Z) reproduce A_norm @ Z?"""
    rng = np.random.default_rng(12345)
    Z = rng.standard_normal((1, N, 4)).astype(np.float32)
    want = np.einsum('ij,bjf->bif', A_norm, Z)
    U = d[None, :, None] * Z
    got = d[None, :, None] * _np_ents(_np_tri_shift(U), U)
    scale = np.abs(want).max() + 1e-30
    return np.abs(want - got).max() / scale < 1e-4


def _d2_pattern_matches(d):
    d2 = (d * d).astype(np.float32)
    e = np.full(N, ONE_THIRD, np.float32)
    for (start, stop, step), scale in FIX_GROUPS:
        e[start:stop:step] *= np.float32(scale)
    return np.allclose(d2, e, rtol=3e-5, atol=1e-7)


# ---------------------------------------------------------------------------
# v4 fast-path program
# ---------------------------------------------------------------------------

def _build_program_v4(warmup=26, dve_h1=(1, 5), debug=False):
    nc = bass.Bass()

    xq_d = nc.declare_dram_parameter("xq", [128, 2 * N], BF16, isOutput=False)
    w1q_d = nc.declare_dram_parameter("w1q", [128, 256], BF16, isOutput=False)
    w2pc_d = nc.declare_dram_parameter("w2pc", [128, 288], BF16, isOutput=False)
    outq_d = nc.declare_dram_parameter("outq", [2, 128, N], BF16, isOutput=True)
    if debug:
        dbg_d = {nm: nc.declare_dram_parameter(f"dbg_{nm}", shp, BF16,
                                               isOutput=True)
                 for nm, shp in [("gp", [128, 2 * N]), ("h1", [128, 2 * IPC * N]),
                                 ("u2", [128, IPC * N]), ("s2", [128, IPC * N]),
                                 ("u3", [128, 2 * N]), ("s3", [128, 2 * N]),
                                 ("m4", [128, 2 * N])]}

    def tri2(S, U, c0, eng=None):
        eng = eng or nc.vector
        eng.tensor_tensor(S[:, c0 + 1:c0 + N], U[:, c0 + 1:c0 + N],
                          U[:, c0:c0 + N - 1], op=ADD)
        eng.tensor_copy(S[:, c0:c0 + 1], U[:, c0:c0 + 1])
        eng.tensor_tensor(S[:, c0:c0 + N - 1], S[:, c0:c0 + N - 1],
                          U[:, c0 + 1:c0 + N], op=ADD)

    def ents2(S, U, c0, eng=None):
        eng = eng or nc.vector
        pairs = [
            (S[:, c0 + 256:c0 + 513:256], S[:, c0 + 100:c0 + 201:100], ADD),
            (S[:, c0 + 256:c0 + 513:256], U[:, c0 + 255:c0 + 512:256], SUB),
            (S[:, c0 + 100:c0 + 201:100], U[:, c0 + 256:c0 + 513:256], ADD),
            (S[:, c0 + 100:c0 + 201:100], U[:, c0 + 257:c0 + 514:256], ADD),
            (S[:, c0 + 255:c0 + 512:256], U[:, c0 + 256:c0 + 513:256], SUB),
        ]
        for dst, s_, op in pairs:
            eng.tensor_tensor(dst, dst, s_, op=op)

    def tri_pair(Sv, Uv, i0, i1):
        nc.vector.tensor_tensor(Sv[:, i0:i1, 1:N], Uv[:, i0:i1, 1:N],
                                Uv[:, i0:i1, 0:N - 1], op=ADD)
        nc.vector.tensor_copy(Sv[:, i0:i1, 0:1], Uv[:, i0:i1, 0:1])
        nc.vector.tensor_tensor(Sv[:, i0:i1, 0:N - 1], Sv[:, i0:i1, 0:N - 1],
                                Uv[:, i0:i1, 1:N], op=ADD)

    def ents_pair(Sv, Uv, i0, i1):
        I = slice(i0, i1)
        P = slice(0, 128)
        pairs = [
            (Sv[P, I, 256:513:256], Sv[P, I, 100:201:100], ADD),
            (Sv[P, I, 256:513:256], Uv[P, I, 255:512:256], SUB),
            (Sv[P, I, 100:201:100], Uv[P, I, 256:513:256], ADD),
            (Sv[P, I, 100:201:100], Uv[P, I, 257:514:256], ADD),
            (Sv[P, I, 255:512:256], Uv[P, I, 256:513:256], SUB),
        ]
        for dst, s_, op in pairs:
            nc.vector.tensor_tensor(dst, dst, s_, op=op)

    with TileContext(nc) as tc:
        with (
            tc.tile_pool(name="const", bufs=1) as cpool,
            tc.tile_pool(name="big", bufs=2, space="PSUM") as big,
            tc.tile_pool(name="p4", bufs=1, space="PSUM") as p4,
        ):
            # ---- constant tiles & DMAs ---------------------------------
            wz = cpool.tile([128, 256], BF16)
            nc.gpsimd.memset(wz[:, :], 0.0)
            # d² plane (one group width; identical for both groups), exact
            # values, built by the otherwise-idle gpsimd — no DMA.
            d2w = cpool.tile([128, N], BF16)
            nc.gpsimd.memset(d2w[:, :], ONE_THIRD)
            nc.gpsimd.memset(d2w[:, 0:1], 0.5)
            nc.gpsimd.memset(d2w[:, 255:768:256], 0.5)
            nc.gpsimd.memset(d2w[:, 100:201:100], 0.2)
            nc.gpsimd.memset(d2w[:, 256:513:256], 0.2)
            xpk = cpool.tile([128, 2 * N], BF16)
            nc.sync.dma_start(xpk[:, :], xq_d[:, :])
            w1 = cpool.tile([128, 256], BF16)
            nc.scalar.dma_start(w1[:, :], w1q_d[:, :])
            w2pc = cpool.tile([128, 288], BF16)
            nc.scalar.dma_start(w2pc[:, :], w2pc_d[:, :])

            G = cpool.tile([128, 2 * N], BF16)
            Gp = cpool.tile([128, 2 * N], BF16)
            h1 = cpool.tile([128, 2 * IPC * N], BF16)   # [kh*6144 + i*768 + n]
            u2 = cpool.tile([128, IPC * N], BF16)
            s2 = cpool.tile([128, IPC * N], BF16)
            U2v = u2[:, :].rearrange("p (i n) -> p i n", n=N)
            S2v = s2[:, :].rearrange("p (i n) -> p i n", n=N)
            u3g = [cpool.tile([128, N], BF16, name=f"u3g{g}") for g in range(2)]
            s3g = [cpool.tile([128, N], BF16, name=f"s3g{g}") for g in range(2)]
            m4g = [cpool.tile([128, N], BF16, name=f"m4g{g}") for g in range(2)]
            s4g = [cpool.tile([128, N], BF16, name=f"s4g{g}") for g in range(2)]
            wsink = cpool.tile([128, 1], F32)

            # ---- PE warm-up burst (no DMA dependency: zeros tile) ------
            wps = p4.tile([128, N], F32, tag="p4")
            for _ in range(warmup):
                nc.tensor.matmul(wps[:, 0:256], wz[0:128, 0:128], wz[:, :],
                                 start=True, stop=True)

            # ---- pipeline stages ---------------------------------------
            C1 = 514

            def L1_group0_chunked():
                # columns 0:C1 first so f1_0's first (cs=0:512) matmul can
                # start before the rest of L1 finishes (all ent columns < C1)
                nc.vector.tensor_tensor(G[:, 1:C1], xpk[:, 1:C1],
                                        xpk[:, 0:C1 - 1], op=ADD)
                nc.vector.tensor_copy(G[:, 0:1], xpk[:, 0:1])
                nc.vector.tensor_tensor(G[:, 0:C1], G[:, 0:C1],
                                        xpk[:, 1:C1 + 1], op=ADD)
                ents2(G, xpk, 0)
                nc.vector.tensor_mul(Gp[:, 0:512], d2w[:, 0:512], G[:, 0:512])
                nc.vector.tensor_tensor(G[:, C1:N], xpk[:, C1:N],
                                        xpk[:, C1 - 1:N - 1], op=ADD)
                nc.vector.tensor_tensor(G[:, C1:N - 1], G[:, C1:N - 1],
                                        xpk[:, C1 + 1:N], op=ADD)
                nc.vector.tensor_mul(Gp[:, 512:N], d2w[:, 512:N],
                                     G[:, 512:N])

            def L1_group(g):
                c0 = g * N
                tri2(G, xpk, c0)
                ents2(G, xpk, c0)
                nc.vector.tensor_mul(Gp[:, c0:c0 + N], d2w[:, :],
                                     G[:, c0:c0 + N])

            def f1(i):
                g, j = divmod(i, 4)
                T = big.tile([128, 2 * N], F32, tag="big", name=f"t1_{i}")
                # psum chunks must not cross 512-f32 bank boundaries:
                # kh0 occupies tile cols 0:768 (chunks 512+256), kh1 cols
                # 768:1536 (chunks 256+512).
                chunks = (((0, 512), (512, 256)), ((0, 256), (256, 512)))
                for kh in range(2):
                    for cs, w in chunks[kh]:
                        nc.tensor.matmul(
                            T[:, kh * N + cs:kh * N + cs + w],
                            w1[32 * j:32 * j + 6, kh * 128:(kh + 1) * 128],
                            Gp[32 * j:32 * j + 6, g * N + cs:g * N + cs + w],
                            start=True, stop=True, tile_position=(32 * j, 0))
                return T

            def f1_evac(i, T):
                for kh in range(2):
                    dst = h1[:, kh * IPC * N + i * N:(kh * IPC + i + 1) * N]
                    src = T[:, kh * N:(kh + 1) * N]
                    if i in dve_h1 and kh == 1:
                        nc.vector.tensor_scalar(dst, src, 0.0, None, op0=MAX)
                    else:
                        nc.scalar.activation(dst, src, RELU)

            def f2(q):
                i = 2 * q
                P = big.tile([128, 2 * N], F32, tag="big", name=f"t2_{q}")
                for c in (0, 512, 1024):
                    for kh in range(2):
                        nc.tensor.matmul(
                            P[:, c:c + 512],
                            w2pc[:, kh * 128:(kh + 1) * 128],
                            h1[:, kh * IPC * N + i * N + c:
                               kh * IPC * N + i * N + c + 512],
                            start=(kh == 0), stop=(kh == 1))
                return P

            def u2_evac(q, P):
                i = 2 * q
                if q == 3:
                    # last pair is tail-critical: halve evac latency by
                    # splitting across ACT and DVE
                    nc.scalar.copy(u2[:, i * N:(i + 1) * N], P[:, 0:N])
                    nc.vector.tensor_copy(u2[:, (i + 1) * N:(i + 2) * N],
                                          P[:, N:2 * N])
                else:
                    nc.scalar.copy(u2[:, i * N:(i + 2) * N], P[:, 0:2 * N])

            def agg2(q):
                i = 2 * q
                tri_pair(S2v, U2v, i, i + 2)
                ents_pair(S2v, U2v, i, i + 2)
                sl = s2[:, i * N:(i + 2) * N]
                nc.vector.tensor_scalar(sl, sl, 0.0, None, op0=MAX)

            def f4(i, Q):
                g, j = divmod(i, 4)
                for cs, w in ((0, 512), (512, 256)):
                    nc.tensor.matmul(
                        Q[32 * j:32 * j + 32, cs:cs + w],
                        w2pc[:, 256:288],
                        s2[:, i * N + cs:i * N + cs + w],
                        start=True, stop=True, tile_position=(0, 32 * j))

            def u3_evac(g, Q):
                nc.vector.tensor_mul(u3g[g][:, :], d2w[:, :], Q[:, 0:N])

            def tail_S3(g):
                tri2(s3g[g], u3g[g], 0)
                ents2(s3g[g], u3g[g], 0)

            def tail_m4(g):
                nc.vector.tensor_mul(m4g[g][:, :], d2w[:, :], s3g[g][:, :])

            def tail_S4(g):
                tri2(s4g[g], m4g[g], 0)
                ents2(s4g[g], m4g[g], 0)

            def out_dma(g):
                nc.sync.dma_start(outq_d[g], s4g[g][:, :])

            # ---- schedule ----------------------------------------------
            L1_group0_chunked()
            T0 = f1(0)
            f1_evac(0, T0)
            T1 = f1(1)
            L1_group(1)
            nc.vector.tensor_copy(wsink[:, :], wps[:, 0:1])
            f1_evac(1, T1)
            P0 = f2(0)
            u2_evac(0, P0)
            T2 = f1(2)
            f1_evac(2, T2)
            agg2(0)
            T3 = f1(3)
            f1_evac(3, T3)
            P1 = f2(1)
            u2_evac(1, P1)
            Qg0 = p4.tile([128, N], F32, tag="p4", name="q0")
            f4(0, Qg0)
            f4(1, Qg0)
            agg2(1)
            T4 = f1(4)
            f1_evac(4, T4)
            T5 = f1(5)
            f1_evac(5, T5)
            f4(2, Qg0)
            f4(3, Qg0)
            u3_evac(0, Qg0)
            P2 = f2(2)
            u2_evac(2, P2)
            tail_S3(0)
            tail_m4(0)
            T6 = f1(6)
            f1_evac(6, T6)
            agg2(2)
            T7 = f1(7)
            f1_evac(7, T7)
            tail_S4(0)
            out_dma(0)
            P3 = f2(3)
            u2_evac(3, P3)
            agg2(3)
            Qg1 = p4.tile([128, N], F32, tag="p4", name="q1")
            for i in range(4, 8):
                f4(i, Qg1)
            u3_evac(1, Qg1)
            tail_S3(1)
            tail_m4(1)
            tail_S4(1)
            out_dma(1)

            if debug:
                nc.sync.dma_start(dbg_d["gp"][:, :], Gp[:, :])
                nc.sync.dma_start(dbg_d["h1"][:, :], h1[:, :])
                nc.sync.dma_start(dbg_d["u2"][:, :], u2[:, :])
                nc.sync.dma_start(dbg_d["s2"][:, :], s2[:, :])
                for g in range(2):
                    nc.sync.dma_start(dbg_d["u3"][:, g * N:(g + 1) * N],
                                      u3g[g][:, :])
                    nc.sync.dma_start(dbg_d["s3"][:, g * N:(g + 1) * N],
                                      s3g[g][:, :])
                    nc.sync.dma_start(dbg_d["m4"][:, g * N:(g + 1) * N],
                                      m4g[g][:, :])

    return nc


# ---------------------------------------------------------------------------
# v1 dense fallback (bias / unexpected adjacency)
# ---------------------------------------------------------------------------

def _build_program_v1(with_bias: bool):
    nc = bass.Bass()
    KT = N // 128

    xT_d = nc.declare_dram_parameter("xT", [IPC, 6, N], F32, isOutput=False)
    anT_d = nc.declare_dram_parameter("anT", [N, N], F32, isOutput=False)
    a2T_d = nc.declare_dram_parameter("a2T", [N, N], F32, isOutput=False)
    w1T_d = nc.declare_dram_parameter("w1T", [6, 256], F32, isOutput=False)
    w2Tp_d = nc.declare_dram_parameter("w2Tp", [128, 256], F32, isOutput=False)
    w34T_d = nc.declare_dram_parameter("w34T", [128, 3], F32, isOutput=False)
    if with_bias:
        p1t_d = nc.declare_dram_parameter("p1t", [128, 2 * N], F32, isOutput=False)
        p2t_d = nc.declare_dram_parameter("p2t", [128, N], F32, isOutput=False)
        cpt_d = nc.declare_dram_parameter("cpt", [3 * IPC, N], F32, isOutput=False)
    out_d = nc.declare_dram_parameter("outp", [3 * IPC, N], F32, isOutput=True)

    with TileContext(nc) as tc:
        with (
            tc.tile_pool(name="const", bufs=1) as cpool,
            tc.tile_pool(name="acts", bufs=2) as apool,
            tc.tile_pool(name="psf", bufs=2, space="PSUM") as psf,
            tc.tile_pool(name="psa", bufs=3, space="PSUM") as psa,
        ):
            anT = cpool.tile([128, KT * N], F32)
            nc.sync.dma_start(
                anT[:, :].rearrange("p (k j) -> p k j", j=N),
                anT_d[:, :].rearrange("(k p) j -> p k j", p=128))
            a2T = cpool.tile([128, KT * N], F32)
            nc.sync.dma_start(
                a2T[:, :].rearrange("p (k j) -> p k j", j=N),
                a2T_d[:, :].rearrange("(k p) j -> p k j", p=128))
            w1T = cpool.tile([6, 256], F32)
            nc.sync.dma_start(w1T[:, :], w1T_d[:, :])
            w2Tp = cpool.tile([128, 256], F32)
            nc.sync.dma_start(w2Tp[:, :], w2Tp_d[:, :])
            w34T = cpool.tile([128, 3], F32)
            nc.sync.dma_start(w34T[:, :], w34T_d[:, :])
            if with_bias:
                p1t = cpool.tile([128, 2 * N], F32)
                nc.sync.dma_start(p1t[:, :], p1t_d[:, :])
                p2t = cpool.tile([128, N], F32)
                nc.sync.dma_start(p2t[:, :], p2t_d[:, :])
                cpt = cpool.tile([3 * IPC, N], F32)
                nc.sync.dma_start(cpt[:, :], cpt_d[:, :])

            z34 = cpool.tile([128, KT * 3 * IPC], F32)

            for it in range(IPC):
                xT = apool.tile([6, N], F32, tag="xT")
                nc.sync.dma_start(xT[:, :], xT_d[it])

                z1 = apool.tile([128, KT * 256], F32, tag="z1")
                for m in range(KT):
                    ps = psf.tile([128, 256], F32, tag="feat")
                    nc.tensor.matmul(
                        ps[:, :], xT[:, m * 128:(m + 1) * 128], w1T[:, :],
                        start=True, stop=True,
                    )
                    nc.vector.tensor_copy(z1[:, m * 256:(m + 1) * 256], ps[:, :])

                h1t = apool.tile([128, 2 * N], F32, tag="h1t")
                for fh in range(2):
                    for ns in range(2):
                        ps = psa.tile([128, 384], F32, tag="agg")
                        for k in range(KT):
                            nc.tensor.matmul(
                                ps[:, :],
                                z1[:, k * 256 + fh * 128: k * 256 + fh * 128 + 128],
                                anT[:, k * N + ns * 384: k * N + ns * 384 + 384],
                                start=(k == 0), stop=(k == KT - 1),
                            )
                        dst = h1t[:, fh * N + ns * 384: fh * N + ns * 384 + 384]
                        if with_bias:
                            nc.vector.tensor_tensor(
                                dst, ps[:, :],
                                p1t[:, fh * N + ns * 384: fh * N + ns * 384 + 384],
                                op=ADD,
                            )
                            nc.scalar.activation(dst, dst, RELU)
                        else:
                            nc.scalar.activation(dst, ps[:, :], RELU)

                z2 = apool.tile([128, KT * 128], F32, tag="z2")
                for m in range(KT):
                    ps = psf.tile([128, 128], F32, tag="feat")
                    for kh in range(2):
                        nc.tensor.matmul(
                            ps[:, :],
                            h1t[:, kh * N + m * 128: kh * N + m * 128 + 128],
                            w2Tp[:, kh * 128:(kh + 1) * 128],
                            start=(kh == 0), stop=(kh == 1),
                        )
                    nc.vector.tensor_copy(z2[:, m * 128:(m + 1) * 128], ps[:, :])

                h2t = apool.tile([128, N], F32, tag="h2t")
                for ns in range(2):
                    ps = psa.tile([128, 384], F32, tag="agg")
                    for k in range(KT):
                        nc.tensor.matmul(
                            ps[:, :],
                            z2[:, k * 128:(k + 1) * 128],
                            anT[:, k * N + ns * 384: k * N + ns * 384 + 384],
                            start=(k == 0), stop=(k == KT - 1),
                        )
                    dst = h2t[:, ns * 384: ns * 384 + 384]
                    if with_bias:
                        nc.vector.tensor_tensor(
                            dst, ps[:, :], p2t[:, ns * 384: ns * 384 + 384],
                            op=ADD,
                        )
                        nc.scalar.activation(dst, dst, RELU)
                    else:
                        nc.scalar.activation(dst, ps[:, :], RELU)

                for m in range(KT):
                    ps = psf.tile([128, 3], F32, tag="feat")
                    nc.tensor.matmul(
                        ps[:, :], h2t[:, m * 128:(m + 1) * 128], w34T[:, :],
                        start=True, stop=True,
                    )
                    base = m * 3 * IPC + it * 3
                    nc.vector.tensor_copy(z34[:, base: base + 3], ps[:, :])

            outT = cpool.tile([3 * IPC, N], F32)
            for ns in range(2):
                ps = psa.tile([3 * IPC, 384], F32, tag="agg")
                for k in range(KT):
                    nc.tensor.matmul(
                        ps[:, :],
                        z34[:, k * 3 * IPC:(k + 1) * 3 * IPC],
                        a2T[:, k * N + ns * 384: k * N + ns * 384 + 384],
                        start=(k == 0), stop=(k == KT - 1),
                    )
                dst = outT[:, ns * 384: ns * 384 + 384]
                if with_bias:
                    nc.vector.tensor_tensor(
                        dst, ps[:, :], cpt[:, ns * 384: ns * 384 + 384],
                        op=ADD,
                    )
                else:
                    nc.vector.tensor_copy(dst, ps[:, :])
            nc.sync.dma_start(out_d[:, :], outT[:, :])

    return nc


def kernel(x, inputs, adjacency, W1, b1, W2, b2, W3, b3, W4, b4,
           parent_sel, child1_sel, child2_sel):
    global LAST_RUN_INFO
    x = np.asarray(x, np.float32)
    inp = np.asarray(inputs, np.float32)
    A = np.asarray(adjacency, np.float32)
    W1 = np.asarray(W1, np.float32); b1 = np.asarray(b1, np.float32)
    W2 = np.asarray(W2, np.float32); b2 = np.asarray(b2, np.float32)
    W3 = np.asarray(W3, np.float32); b3 = np.asarray(b3, np.float32)
    W4 = np.asarray(W4, np.float32); b4 = np.asarray(b4, np.float32)
    parent_sel = np.asarray(parent_sel, np.int64)
    child1_sel = np.asarray(child1_sel, np.int64)
    child2_sel = np.asarray(child2_sel, np.int64)

    clamp_rows = np.concatenate([
        parent_sel, NV + child1_sel, 2 * NV + child2_sel,
    ]).astype(np.int64)

    x0 = x.copy()
    x0[:, clamp_rows, 0:3] = inp[:, clamp_rows, :]

    deg = A.sum(axis=-1)
    deg_safe = np.where(deg == 0, np.float32(1.0), deg)
    d = np.where(deg == 0, np.float32(0.0),
                 deg_safe ** np.float32(-0.5)).astype(np.float32)
    A_norm = (A * d[:, None] * d[None, :]).astype(np.float32)

    with_bias = bool(np.any(b1) or np.any(b2) or np.any(b3) or np.any(b4))
    use_v4 = ((not with_bias) and _structure_matches(A_norm, d)
              and _d2_pattern_matches(d))

    trace = os.environ.get("KERNEL_TRACE", "") == "1"

    if use_v4:
        bf = ml_dtypes.bfloat16
        Xd = (d[None, :, None] * x0).astype(bf)                   # (B, N, 6)
        # xq[core, 32j+f, g*N + n] = Xd[core*8 + g*4 + j, n, f]
        Xr = Xd.reshape(NCORES, 2, 4, N, 6).transpose(0, 2, 4, 1, 3)
        xq = np.zeros((NCORES, 4, 32, 2 * N), bf)
        xq[:, :, 0:6, :] = Xr.reshape(NCORES, 4, 6, 2 * N)
        xq = xq.reshape(NCORES, 128, 2 * N)

        w1q = np.zeros((4, 32, 256), bf)
        w1q[:, 0:6, :] = W1.T.astype(bf)[None]
        w1q = np.ascontiguousarray(w1q.reshape(128, 256))
        w2pc = np.zeros((128, 288), bf)
        w2pc[:, 0:256] = np.ascontiguousarray(
            W2.T.reshape(2, 128, 128).transpose(1, 0, 2).reshape(128, 256)
        ).astype(bf)
        w2pc[:, 256:259] = (W3.T @ W4.T).astype(bf)

        nc = _build_program_v4()
        _split_multi_waits(nc)
        in_maps = [{"xq": xq[c], "w1q": w1q, "w2pc": w2pc}
                   for c in range(NCORES)]

        res = run_bass_kernel_spmd(nc, in_maps, list(range(NCORES)),
                                   trace=trace)
        LAST_RUN_INFO = {
            "exec_time_ns": res.exec_time_ns,
            "mean_exec_time_ns": res.mean_exec_time_ns,
            "max_exec_time_core_id": res.max_exec_time_core_id,
        }

        out = np.empty((B, N, 3), np.float32)
        for c in range(NCORES):
            o = np.asarray(res.results[c]["outq"], bf).astype(np.float32)
            # o[g, 32j+s, n] -> out[c*8 + g*4 + j, n, s]
            oi = o.reshape(2, 4, 32, N)[:, :, 0:3, :]     # (g, j, s, n)
            arr = oi.transpose(0, 1, 3, 2).reshape(IPC, N, 3)
            out[c * IPC:(c + 1) * IPC] = arr
        out *= d[None, :, None]
    else:
        AnT = np.ascontiguousarray(A_norm.T)
        A2T = np.ascontiguousarray((A_norm @ A_norm).T.astype(np.float32))
        W1T = np.ascontiguousarray(W1.T)
        W2Tp = np.ascontiguousarray(
            W2.T.reshape(2, 128, 128).transpose(1, 0, 2).reshape(128, 256))
        W34T = np.ascontiguousarray(W3.T @ W4.T)

        extra = {}
        if with_bias:
            s = A_norm.sum(axis=1).astype(np.float32)
            s2 = (A_norm @ s).astype(np.float32)
            p1t = np.einsum('f,n->fn', b1, s).astype(np.float32)
            p1t = p1t.reshape(2, 128, N).transpose(1, 0, 2).reshape(128, 2 * N)
            p2t = np.einsum('f,n->fn', b2, s).astype(np.float32)
            cp = (np.einsum('f,n->fn', W4 @ b3, s2) +
                  np.einsum('f,n->fn', b4, s)).astype(np.float32)
            cpt = np.tile(cp, (IPC, 1)).astype(np.float32)
            extra = {"p1t": np.ascontiguousarray(p1t),
                     "p2t": np.ascontiguousarray(p2t),
                     "cpt": np.ascontiguousarray(cpt)}

        xT_all = np.ascontiguousarray(
            x0.transpose(0, 2, 1).reshape(NCORES, IPC, 6, N))

        nc = _build_program_v1(with_bias)
        _split_multi_waits(nc)

        in_maps = []
        for c in range(NCORES):
            m = {
                "xT": xT_all[c], "anT": AnT, "a2T": A2T,
                "w1T": W1T, "w2Tp": W2Tp, "w34T": W34T,
            }
            m.update(extra)
            in_maps.append(m)

        res = run_bass_kernel_spmd(nc, in_maps, list(range(NCORES)),
                                   trace=trace)
        LAST_RUN_INFO = {
            "exec_time_ns": res.exec_time_ns,
            "mean_exec_time_ns": res.mean_exec_time_ns,
            "max_exec_time_core_id": res.max_exec_time_core_id,
        }

        out = np.empty((B, N, 3), np.float32)
        for c in range(NCORES):
            o = res.results[c]["outp"]
            for it in range(IPC):
                out[c * IPC + it] = o[it * 3:(it + 1) * 3, :].T
    out[:, clamp_rows, :] = inp[:, clamp_rows, :]
    return out


# revision 27
# speedup vs baseline: 1.1592x; 1.1592x over previous
"""Trainium2 Bass kernel for BatchedGNNModel (4-layer GCN over 3-rod chain graph).

Contract: kernel(**inputs) takes FULL unsharded inputs (as produced by
setup_inputs) and returns the FULL (64, 768, 3) float32 output.

Sharding: pure data parallel over batch — 8 items per NeuronCore on 8 cores,
identical SPMD program, weights replicated (marshaled on host).

v4 fast-path algorithm (zero biases, expected adjacency structure):
  A_norm = D·M·D with D = diag(d), d = deg^-1/2, M = tridiagonal-support ones
  + ~10 coefficient-1 sparse corrections. One application of M is
  S = tri_shift(U) + ents(U). d² is 1/3 everywhere except 8 columns
  ({0,255,511,767}: 1/2, {100,200,256,512}: 1/5), so every d²⊙ plane multiply
  is a tensor_scalar ×(1/3) plus 4 tiny strided column-fix multiplies — no
  d² plane in SBUF or DMA at all.

  Folded chain (relu is positively homogeneous, feature ops commute with
  node-diagonal scales):
    Gp = d²⊙(M (d⊙x))          [L1: DVE tri+ents+scale on packed x]
    h1 = relu(Gp @ W1ᵀ)        [f1: PE per-item 6-contract matmuls, ACT/DVE evac]
    u2 = h1 @ W2ᵀ              [f2: PE 128-contract matmuls per item pair]
    s2 = relu(M u2)            [agg2: DVE tri+ents+relu per pair]
    u3 = d²⊙(s2 @ WC)          [f4: PE item-packed 32-col matmuls, WC = W3ᵀW4ᵀ]
    out = d ⊙ M (d²⊙(M u3))    [tail per 4-item group: S3, m4, S4; d⊙ on host]

  Layout: items packed 4 per group at partition stride 32, features 0:6 of
  each 32-band; 2 groups as column blocks of 768. All activations bf16.
  Per-item software pipeline: f1(i)→f2(pair)→agg2(pair)→f4(i), with group 0's
  whole tail + output DMA issued mid-kernel so only group 1's tail is exposed.
  Input DMAs are row-sparse (only the 6 used partitions per band) and split
  across the two HWDGE queues (sync: x, scalar: weights) so descriptor
  writing parallelizes and first compute starts ~3us earlier.

Fallback path (nonzero biases or unexpected adjacency/d²): v1 dense program.

This image's walrus accepts only one sync-wait slot per instruction, so a
post-pass splits Tile's multi-wait instructions into single-wait NoOps.
"""

import os
import sys

import numpy as np

sys.path.insert(0, "/opt/trn_rl_repo")

import ml_dtypes
import concourse.bass as bass
import concourse.mybir as mybir
import concourse.tile as _tile_mod
from concourse.tile import TileContext
from concourse.vector_clock import ScopedClock
from concourse.bass_utils import run_bass_kernel_spmd


def _patched_drain_and_barrier(self, tick_clock, wait_clock):
    """The nix walrus in this image only supports one sync-wait slot on a
    Drain; Tile's kernel-tail drain carries one wait per ticked semaphore.
    Split the extra waits onto single-wait nops on the same (sync) engine —
    program order makes this equivalent before the all-engine barrier."""
    drain_inst = self.nc.sync.drain()
    wait_clock.add_sem_waits(
        drain_inst.ins, ScopedClock({None: tick_clock.global_clock}))
    waits = list(drain_inst.ins.sync_info.on_wait)
    if len(waits) > 1:
        import bass_rust
        drain_inst.ins.sync_info.on_wait = [waits[0]]
        for w in waits[1:]:
            nop = self.nc.sync.nop(nofuse=True)
            si = nop.ins.sync_info
            if si is None:
                nop.ins.sync_info = bass_rust.SyncInfo(on_wait=[w], on_update=[])
            else:
                si.on_wait = [w]
    self.nc.all_engine_barrier()
    assert self.sems is not None
    popped = self.nc._tile_sem_poison_stack.pop()
    assert popped is self._sem_poison
    self.nc.clear_and_free_semaphores(list(self.sems.allocated().values()))
    self.nc.all_engine_barrier()


_tile_mod.TileContext._drain_and_barrier = _patched_drain_and_barrier


def _split_multi_waits(nc):
    """This image's walrus supports a single sync-wait slot per instruction.
    Hoist all-but-one wait of any multi-wait instruction onto single-wait
    NoOps on the same engine, placed immediately before it (same per-engine
    program order => equivalent synchronization)."""
    for f in nc.m.functions:
        for bb in f.blocks:
            insts = list(bb.instructions)
            if not any(ins.sync_info and len(ins.sync_info.on_wait) > 1
                       for ins in insts):
                continue
            new = []
            for ins in insts:
                si = ins.sync_info
                if si is not None and len(si.on_wait) > 1:
                    waits = list(si.on_wait)
                    for w in waits[:-1]:
                        new.append(mybir.InstNoOp(
                            name=nc.get_next_instruction_name(),
                            sync_info=mybir.SyncInfo(on_wait=[w], on_update=[]),
                            bass_nofuse=True,
                            engine=ins.engine,
                        ))
                    si.on_wait = [waits[-1]]
                new.append(ins)
            bb.instructions = new


def _ensure_ntff_hook():
    """The agent image's antenv lacks axon_hooks; bass_utils imports it when
    trace=True. Install a shim and, if possible, the real ctypes profiler."""
    import types
    try:
        import antenv.axon_hooks  # noqa: F401
        return
    except Exception:
        pass
    try:
        import antenv
        mod = types.ModuleType("antenv.axon_hooks")
        state = {"h": None}
        mod.set_axon_ntff_profile_hook = lambda h: state.__setitem__("h", h)
        mod.get_axon_ntff_profile_hook = lambda: state["h"]
        sys.modules["antenv.axon_hooks"] = mod
        antenv.axon_hooks = mod
        try:
            from trn_agent_boot.trn_boot import _ntff_profile_via_ctypes
            mod.set_axon_ntff_profile_hook(
                _ntff_profile_via_ctypes("/opt/axon/libaxon_pjrt.so"))
        except Exception:
            pass
    except Exception:
        pass


_ensure_ntff_hook()

F32 = mybir.dt.float32
BF16 = mybir.dt.bfloat16
RELU = mybir.ActivationFunctionType.Relu
ADD = mybir.AluOpType.add
SUB = mybir.AluOpType.subtract
MULT = mybir.AluOpType.mult
MAX = mybir.AluOpType.max

B = 64
NV = 256
N = 3 * NV  # 768
NCORES = 8
IPC = B // NCORES  # 8 items per core

ONE_THIRD = float(np.float32(1.0) / np.float32(3.0))

LAST_RUN_INFO = {}

# Sparse corrections for one M application, coefficient-1 form, order-safe:
# (dst_col, 'S'|'U', src_col, op). S reads must precede writes to their col.
ENT_OPS = [
    (256, 'S', 100, ADD), (512, 'S', 200, ADD),
    (256, 'U', 255, SUB), (512, 'U', 511, SUB),
    (100, 'U', 256, ADD), (100, 'U', 257, ADD),
    (200, 'U', 512, ADD), (200, 'U', 513, ADD),
    (255, 'U', 256, SUB), (511, 'U', 512, SUB),
]

# d² = 1/3 everywhere except: ×3/2 at {0,255,511,767}, ×3/5 at {100,200,256,512}
FIX_GROUPS = [
    ((0, 1, 1), 1.5),
    ((255, 768, 256), 1.5),
    ((100, 201, 100), 0.6),
    ((256, 513, 256), 0.6),
]


def _np_tri_shift(U):
    S = U.copy()
    S[..., 1:, :] += U[..., :-1, :]
    S[..., :-1, :] += U[..., 1:, :]
    return S


def _np_ents(S, U):
    for (j, kind, k, op) in ENT_OPS:
        src = (S if kind == 'S' else U)[..., k, :].copy()
        if op is ADD:
            S[..., j, :] += src
        else:
            S[..., j, :] -= src
    return S


def _structure_matches(A_norm, d):
    """Does d ⊙ (tri+ents)(d ⊙ Z) reproduce A_norm @ Z?"""
    rng = np.random.default_rng(12345)
    Z = rng.standard_normal((1, N, 4)).astype(np.float32)
    want = np.einsum('ij,bjf->bif', A_norm, Z)
    U = d[None, :, None] * Z
    got = d[None, :, None] * _np_ents(_np_tri_shift(U), U)
    scale = np.abs(want).max() + 1e-30
    return np.abs(want - got).max() / scale < 1e-4


def _d2_pattern_matches(d):
    d2 = (d * d).astype(np.float32)
    e = np.full(N, ONE_THIRD, np.float32)
    for (start, stop, step), scale in FIX_GROUPS:
        e[start:stop:step] *= np.float32(scale)
    return np.allclose(d2, e, rtol=3e-5, atol=1e-7)


# ---------------------------------------------------------------------------
# v4 fast-path program
# ---------------------------------------------------------------------------

def _build_program_v4(warmup=16, dve_h1=(), debug=False):
    nc = bass.Bass()

    xq_d = nc.declare_dram_parameter("xq", [128, 2 * N], BF16, isOutput=False)
    w1q_d = nc.declare_dram_parameter("w1q", [128, 256], BF16, isOutput=False)
    w2pc_d = nc.declare_dram_parameter("w2pc", [128, 288], BF16, isOutput=False)
    outq_d = nc.declare_dram_parameter("outq", [2, 128, N], BF16, isOutput=True)
    if debug:
        dbg_d = {nm: nc.declare_dram_parameter(f"dbg_{nm}", shp, BF16,
                                               isOutput=True)
                 for nm, shp in [("gp", [128, 2 * N]), ("h1", [128, 2 * IPC * N]),
                                 ("u2", [128, IPC * N]), ("s2", [128, IPC * N]),
                                 ("u3", [128, 2 * N]), ("s3", [128, 2 * N]),
                                 ("m4", [128, 2 * N])]}

    def tri2(S, U, c0, eng=None):
        eng = eng or nc.vector
        eng.tensor_tensor(S[:, c0 + 1:c0 + N], U[:, c0 + 1:c0 + N],
                          U[:, c0:c0 + N - 1], op=ADD)
        eng.tensor_copy(S[:, c0:c0 + 1], U[:, c0:c0 + 1])
        eng.tensor_tensor(S[:, c0:c0 + N - 1], S[:, c0:c0 + N - 1],
                          U[:, c0 + 1:c0 + N], op=ADD)

    def ents2(S, U, c0, eng=None):
        eng = eng or nc.vector
        pairs = [
            (S[:, c0 + 256:c0 + 513:256], S[:, c0 + 100:c0 + 201:100], ADD),
            (S[:, c0 + 256:c0 + 513:256], U[:, c0 + 255:c0 + 512:256], SUB),
            (S[:, c0 + 100:c0 + 201:100], U[:, c0 + 256:c0 + 513:256], ADD),
            (S[:, c0 + 100:c0 + 201:100], U[:, c0 + 257:c0 + 514:256], ADD),
            (S[:, c0 + 255:c0 + 512:256], U[:, c0 + 256:c0 + 513:256], SUB),
        ]
        for dst, s_, op in pairs:
            eng.tensor_tensor(dst, dst, s_, op=op)

    def tri_pair(Sv, Uv, i0, i1):
        nc.vector.tensor_tensor(Sv[:, i0:i1, 1:N], Uv[:, i0:i1, 1:N],
                                Uv[:, i0:i1, 0:N - 1], op=ADD)
        nc.vector.tensor_copy(Sv[:, i0:i1, 0:1], Uv[:, i0:i1, 0:1])
        nc.vector.tensor_tensor(Sv[:, i0:i1, 0:N - 1], Sv[:, i0:i1, 0:N - 1],
                                Uv[:, i0:i1, 1:N], op=ADD)

    def ents_pair(Sv, Uv, i0, i1):
        I = slice(i0, i1)
        P = slice(0, 128)
        pairs = [
            (Sv[P, I, 256:513:256], Sv[P, I, 100:201:100], ADD),
            (Sv[P, I, 256:513:256], Uv[P, I, 255:512:256], SUB),
            (Sv[P, I, 100:201:100], Uv[P, I, 256:513:256], ADD),
            (Sv[P, I, 100:201:100], Uv[P, I, 257:514:256], ADD),
            (Sv[P, I, 255:512:256], Uv[P, I, 256:513:256], SUB),
        ]
        for dst, s_, op in pairs:
            nc.vector.tensor_tensor(dst, dst, s_, op=op)

    with TileContext(nc) as tc:
        with (
            tc.tile_pool(name="const", bufs=1) as cpool,
            tc.tile_pool(name="big", bufs=2, space="PSUM") as big,
            tc.tile_pool(name="p4", bufs=1, space="PSUM") as p4,
        ):
            # ---- constant tiles & DMAs ---------------------------------
            wz = cpool.tile([128, 256], BF16)
            nc.gpsimd.memset(wz[:, :], 0.0)
            # d² plane (one group width; identical for both groups), exact
            # values, built by the otherwise-idle gpsimd — no DMA.
            d2w = cpool.tile([128, N], BF16)
            nc.gpsimd.memset(d2w[:, :], ONE_THIRD)
            nc.gpsimd.memset(d2w[:, 0:1], 0.5)
            nc.gpsimd.memset(d2w[:, 255:768:256], 0.5)
            nc.gpsimd.memset(d2w[:, 100:201:100], 0.2)
            nc.gpsimd.memset(d2w[:, 256:513:256], 0.2)
            xpk = cpool.tile([128, 2 * N], BF16)
            nc.sync.dma_start(xpk[:, :], xq_d[:, :])
            w1 = cpool.tile([128, 256], BF16)
            nc.scalar.dma_start(w1[:, :], w1q_d[:, :])
            w2pc = cpool.tile([128, 288], BF16)
            nc.scalar.dma_start(w2pc[:, :], w2pc_d[:, :])

            G = cpool.tile([128, 2 * N], BF16)
            Gp = cpool.tile([128, 2 * N], BF16)
            h1 = cpool.tile([128, 2 * IPC * N], BF16)   # [kh*6144 + i*768 + n]
            u2 = cpool.tile([128, IPC * N], BF16)
            s2 = cpool.tile([128, IPC * N], BF16)
            U2v = u2[:, :].rearrange("p (i n) -> p i n", n=N)
            S2v = s2[:, :].rearrange("p (i n) -> p i n", n=N)
            u3g = [cpool.tile([128, N], BF16, name=f"u3g{g}") for g in range(2)]
            s3g = [cpool.tile([128, N], BF16, name=f"s3g{g}") for g in range(2)]
            m4g = [cpool.tile([128, N], BF16, name=f"m4g{g}") for g in range(2)]
            s4g = [cpool.tile([128, N], BF16, name=f"s4g{g}") for g in range(2)]
            wsink = cpool.tile([128, 1], F32)

            # ---- PE warm-up burst (no DMA dependency: zeros tile) ------
            wps = p4.tile([128, N], F32, tag="p4")
            for _ in range(warmup):
                nc.tensor.matmul(wps[:, 0:256], wz[0:128, 0:128], wz[:, :],
                                 start=True, stop=True)

            # ---- pipeline stages ---------------------------------------
            C1 = 514

            def L1_group0_chunked():
                # columns 0:C1 first so f1_0's first (cs=0:512) matmul can
                # start before the rest of L1 finishes (all ent columns < C1)
                nc.vector.tensor_tensor(G[:, 1:C1], xpk[:, 1:C1],
                                        xpk[:, 0:C1 - 1], op=ADD)
                nc.vector.tensor_copy(G[:, 0:1], xpk[:, 0:1])
                nc.vector.tensor_tensor(G[:, 0:C1], G[:, 0:C1],
                                        xpk[:, 1:C1 + 1], op=ADD)
                ents2(G, xpk, 0)
                nc.vector.tensor_mul(Gp[:, 0:512], d2w[:, 0:512], G[:, 0:512])
                nc.vector.tensor_tensor(G[:, C1:N], xpk[:, C1:N],
                                        xpk[:, C1 - 1:N - 1], op=ADD)
                nc.vector.tensor_tensor(G[:, C1:N - 1], G[:, C1:N - 1],
                                        xpk[:, C1 + 1:N], op=ADD)
                nc.vector.tensor_mul(Gp[:, 512:N], d2w[:, 512:N],
                                     G[:, 512:N])

            def L1_group(g):
                c0 = g * N
                tri2(G, xpk, c0)
                ents2(G, xpk, c0)
                nc.vector.tensor_mul(Gp[:, c0:c0 + N], d2w[:, :],
                                     G[:, c0:c0 + N])

            def f1(i):
                g, j = divmod(i, 4)
                T = big.tile([128, 2 * N], F32, tag="big", name=f"t1_{i}")
                # psum chunks must not cross 512-f32 bank boundaries:
                # kh0 occupies tile cols 0:768 (chunks 512+256), kh1 cols
                # 768:1536 (chunks 256+512).
                chunks = (((0, 512), (512, 256)), ((0, 256), (256, 512)))
                for kh in range(2):
                    for cs, w in chunks[kh]:
                        nc.tensor.matmul(
                            T[:, kh * N + cs:kh * N + cs + w],
                            w1[32 * j:32 * j + 6, kh * 128:(kh + 1) * 128],
                            Gp[32 * j:32 * j + 6, g * N + cs:g * N + cs + w],
                            start=True, stop=True, tile_position=(32 * j, 0))
                return T

            def f1_evac(i, T):
                for kh in range(2):
                    dst = h1[:, kh * IPC * N + i * N:(kh * IPC + i + 1) * N]
                    src = T[:, kh * N:(kh + 1) * N]
                    if i in dve_h1 and kh == 1:
                        nc.vector.tensor_scalar(dst, src, 0.0, None, op0=MAX)
                    else:
                        nc.scalar.activation(dst, src, RELU)

            def f2(q):
                i = 2 * q
                P = big.tile([128, 2 * N], F32, tag="big", name=f"t2_{q}")
                for c in (0, 512, 1024):
                    for kh in range(2):
                        nc.tensor.matmul(
                            P[:, c:c + 512],
                            w2pc[:, kh * 128:(kh + 1) * 128],
                            h1[:, kh * IPC * N + i * N + c:
                               kh * IPC * N + i * N + c + 512],
                            start=(kh == 0), stop=(kh == 1))
                return P

            def u2_evac(q, P):
                i = 2 * q
                if q == 3:
                    # last pair is tail-critical: halve evac latency by
                    # splitting across ACT and DVE
                    nc.scalar.copy(u2[:, i * N:(i + 1) * N], P[:, 0:N])
                    nc.vector.tensor_copy(u2[:, (i + 1) * N:(i + 2) * N],
                                          P[:, N:2 * N])
                else:
                    nc.scalar.copy(u2[:, i * N:(i + 2) * N], P[:, 0:2 * N])

            def agg2(q):
                i = 2 * q
                tri_pair(S2v, U2v, i, i + 2)
                ents_pair(S2v, U2v, i, i + 2)
                sl = s2[:, i * N:(i + 2) * N]
                nc.vector.tensor_scalar(sl, sl, 0.0, None, op0=MAX)

            def f4(i, Q):
                g, j = divmod(i, 4)
                for cs, w in ((0, 512), (512, 256)):
                    nc.tensor.matmul(
                        Q[32 * j:32 * j + 32, cs:cs + w],
                        w2pc[:, 256:288],
                        s2[:, i * N + cs:i * N + cs + w],
                        start=True, stop=True, tile_position=(0, 32 * j))

            def u3_evac(g, Q):
                nc.vector.tensor_mul(u3g[g][:, :], d2w[:, :], Q[:, 0:N])

            def tail_S3(g):
                tri2(s3g[g], u3g[g], 0)
                ents2(s3g[g], u3g[g], 0)

            def tail_m4(g):
                nc.vector.tensor_mul(m4g[g][:, :], d2w[:, :], s3g[g][:, :])

            def tail_S4(g):
                tri2(s4g[g], m4g[g], 0)
                ents2(s4g[g], m4g[g], 0)

            def out_dma(g):
                nc.sync.dma_start(outq_d[g], s4g[g][:, :])

            # ---- schedule ----------------------------------------------
            # ACT stream keeps each pair's u2 evac AFTER the next pair's
            # first f1 evacs so f2(q+1) is never blocked behind u2e(q);
            # group-0 tail ops are slotted into DVE's between-pair gaps.
            L1_group0_chunked()
            T0 = f1(0)
            f1_evac(0, T0)
            T1 = f1(1)
            L1_group(1)
            nc.vector.tensor_copy(wsink[:, :], wps[:, 0:1])
            f1_evac(1, T1)
            P0 = f2(0)
            T2 = f1(2)
            f1_evac(2, T2)
            u2_evac(0, P0)
            T3 = f1(3)
            f1_evac(3, T3)
            agg2(0)
            P1 = f2(1)
            Qg0 = p4.tile([128, N], F32, tag="p4", name="q0")
            f4(0, Qg0)
            f4(1, Qg0)
            T4 = f1(4)
            f1_evac(4, T4)
            u2_evac(1, P1)
            T5 = f1(5)
            f1_evac(5, T5)
            agg2(1)
            f4(2, Qg0)
            f4(3, Qg0)
            u3_evac(0, Qg0)
            P2 = f2(2)
            T6 = f1(6)
            f1_evac(6, T6)
            u2_evac(2, P2)
            tail_S3(0)
            tail_m4(0)
            T7 = f1(7)
            f1_evac(7, T7)
            agg2(2)
            tail_S4(0)
            out_dma(0)
            P3 = f2(3)
            u2_evac(3, P3)
            agg2(3)
            Qg1 = p4.tile([128, N], F32, tag="p4", name="q1")
            for i in range(4, 8):
                f4(i, Qg1)
            u3_evac(1, Qg1)
            tail_S3(1)
            tail_m4(1)
            tail_S4(1)
            out_dma(1)

            if debug:
                nc.sync.dma_start(dbg_d["gp"][:, :], Gp[:, :])
                nc.sync.dma_start(dbg_d["h1"][:, :], h1[:, :])
                nc.sync.dma_start(dbg_d["u2"][:, :], u2[:, :])
                nc.sync.dma_start(dbg_d["s2"][:, :], s2[:, :])
                for g in range(2):
                    nc.sync.dma_start(dbg_d["u3"][:, g * N:(g + 1) * N],
                                      u3g[g][:, :])
                    nc.sync.dma_start(dbg_d["s3"][:, g * N:(g + 1) * N],
                                      s3g[g][:, :])
                    nc.sync.dma_start(dbg_d["m4"][:, g * N:(g + 1) * N],
                                      m4g[g][:, :])

    return nc


# ---------------------------------------------------------------------------
# v1 dense fallback (bias / unexpected adjacency)
# ---------------------------------------------------------------------------

def _build_program_v1(with_bias: bool):
    nc = bass.Bass()
    KT = N // 128

    xT_d = nc.declare_dram_parameter("xT", [IPC, 6, N], F32, isOutput=False)
    anT_d = nc.declare_dram_parameter("anT", [N, N], F32, isOutput=False)
    a2T_d = nc.declare_dram_parameter("a2T", [N, N], F32, isOutput=False)
    w1T_d = nc.declare_dram_parameter("w1T", [6, 256], F32, isOutput=False)
    w2Tp_d = nc.declare_dram_parameter("w2Tp", [128, 256], F32, isOutput=False)
    w34T_d = nc.declare_dram_parameter("w34T", [128, 3], F32, isOutput=False)
    if with_bias:
        p1t_d = nc.declare_dram_parameter("p1t", [128, 2 * N], F32, isOutput=False)
        p2t_d = nc.declare_dram_parameter("p2t", [128, N], F32, isOutput=False)
        cpt_d = nc.declare_dram_parameter("cpt", [3 * IPC, N], F32, isOutput=False)
    out_d = nc.declare_dram_parameter("outp", [3 * IPC, N], F32, isOutput=True)

    with TileContext(nc) as tc:
        with (
            tc.tile_pool(name="const", bufs=1) as cpool,
            tc.tile_pool(name="acts", bufs=2) as apool,
            tc.tile_pool(name="psf", bufs=2, space="PSUM") as psf,
            tc.tile_pool(name="psa", bufs=3, space="PSUM") as psa,
        ):
            anT = cpool.tile([128, KT * N], F32)
            nc.sync.dma_start(
                anT[:, :].rearrange("p (k j) -> p k j", j=N),
                anT_d[:, :].rearrange("(k p) j -> p k j", p=128))
            a2T = cpool.tile([128, KT * N], F32)
            nc.sync.dma_start(
                a2T[:, :].rearrange("p (k j) -> p k j", j=N),
                a2T_d[:, :].rearrange("(k p) j -> p k j", p=128))
            w1T = cpool.tile([6, 256], F32)
            nc.sync.dma_start(w1T[:, :], w1T_d[:, :])
            w2Tp = cpool.tile([128, 256], F32)
            nc.sync.dma_start(w2Tp[:, :], w2Tp_d[:, :])
            w34T = cpool.tile([128, 3], F32)
            nc.sync.dma_start(w34T[:, :], w34T_d[:, :])
            if with_bias:
                p1t = cpool.tile([128, 2 * N], F32)
                nc.sync.dma_start(p1t[:, :], p1t_d[:, :])
                p2t = cpool.tile([128, N], F32)
                nc.sync.dma_start(p2t[:, :], p2t_d[:, :])
                cpt = cpool.tile([3 * IPC, N], F32)
                nc.sync.dma_start(cpt[:, :], cpt_d[:, :])

            z34 = cpool.tile([128, KT * 3 * IPC], F32)

            for it in range(IPC):
                xT = apool.tile([6, N], F32, tag="xT")
                nc.sync.dma_start(xT[:, :], xT_d[it])

                z1 = apool.tile([128, KT * 256], F32, tag="z1")
                for m in range(KT):
                    ps = psf.tile([128, 256], F32, tag="feat")
                    nc.tensor.matmul(
                        ps[:, :], xT[:, m * 128:(m + 1) * 128], w1T[:, :],
                        start=True, stop=True,
                    )
                    nc.vector.tensor_copy(z1[:, m * 256:(m + 1) * 256], ps[:, :])

                h1t = apool.tile([128, 2 * N], F32, tag="h1t")
                for fh in range(2):
                    for ns in range(2):
                        ps = psa.tile([128, 384], F32, tag="agg")
                        for k in range(KT):
                            nc.tensor.matmul(
                                ps[:, :],
                                z1[:, k * 256 + fh * 128: k * 256 + fh * 128 + 128],
                                anT[:, k * N + ns * 384: k * N + ns * 384 + 384],
                                start=(k == 0), stop=(k == KT - 1),
                            )
                        dst = h1t[:, fh * N + ns * 384: fh * N + ns * 384 + 384]
                        if with_bias:
                            nc.vector.tensor_tensor(
                                dst, ps[:, :],
                                p1t[:, fh * N + ns * 384: fh * N + ns * 384 + 384],
                                op=ADD,
                            )
                            nc.scalar.activation(dst, dst, RELU)
                        else:
                            nc.scalar.activation(dst, ps[:, :], RELU)

                z2 = apool.tile([128, KT * 128], F32, tag="z2")
                for m in range(KT):
                    ps = psf.tile([128, 128], F32, tag="feat")
                    for kh in range(2):
                        nc.tensor.matmul(
                            ps[:, :],
                            h1t[:, kh * N + m * 128: kh * N + m * 128 + 128],
                            w2Tp[:, kh * 128:(kh + 1) * 128],
                            start=(kh == 0), stop=(kh == 1),
                        )
                    nc.vector.tensor_copy(z2[:, m * 128:(m + 1) * 128], ps[:, :])

                h2t = apool.tile([128, N], F32, tag="h2t")
                for ns in range(2):
                    ps = psa.tile([128, 384], F32, tag="agg")
                    for k in range(KT):
                        nc.tensor.matmul(
                            ps[:, :],
                            z2[:, k * 128:(k + 1) * 128],
                            anT[:, k * N + ns * 384: k * N + ns * 384 + 384],
                            start=(k == 0), stop=(k == KT - 1),
                        )
                    dst = h2t[:, ns * 384: ns * 384 + 384]
                    if with_bias:
                        nc.vector.tensor_tensor(
                            dst, ps[:, :], p2t[:, ns * 384: ns * 384 + 384],
                            op=ADD,
                        )
                        nc.scalar.activation(dst, dst, RELU)
                    else:
                        nc.scalar.activation(dst, ps[:, :], RELU)

                for m in range(KT):
                    ps = psf.tile([128, 3], F32, tag="feat")
                    nc.tensor.matmul(
                        ps[:, :], h2t[:, m * 128:(m + 1) * 128], w34T[:, :],
                        start=True, stop=True,
                    )
                    base = m * 3 * IPC + it * 3
                    nc.vector.tensor_copy(z34[:, base: base + 3], ps[:, :])

            outT = cpool.tile([3 * IPC, N], F32)
            for ns in range(2):
                ps = psa.tile([3 * IPC, 384], F32, tag="agg")
                for k in range(KT):
                    nc.tensor.matmul(
                        ps[:, :],
                        z34[:, k * 3 * IPC:(k + 1) * 3 * IPC],
                        a2T[:, k * N + ns * 384: k * N + ns * 384 + 384],
                        start=(k == 0), stop=(k == KT - 1),
                    )
                dst = outT[:, ns * 384: ns * 384 + 384]
                if with_bias:
                    nc.vector.tensor_tensor(
                        dst, ps[:, :], cpt[:, ns * 384: ns * 384 + 384],
                        op=ADD,
                    )
                else:
                    nc.vector.tensor_copy(dst, ps[:, :])
            nc.sync.dma_start(out_d[:, :], outT[:, :])

    return nc


def kernel(x, inputs, adjacency, W1, b1, W2, b2, W3, b3, W4, b4,
           parent_sel, child1_sel, child2_sel):
    global LAST_RUN_INFO
    x = np.asarray(x, np.float32)
    inp = np.asarray(inputs, np.float32)
    A = np.asarray(adjacency, np.float32)
    W1 = np.asarray(W1, np.float32); b1 = np.asarray(b1, np.float32)
    W2 = np.asarray(W2, np.float32); b2 = np.asarray(b2, np.float32)
    W3 = np.asarray(W3, np.float32); b3 = np.asarray(b3, np.float32)
    W4 = np.asarray(W4, np.float32); b4 = np.asarray(b4, np.float32)
    parent_sel = np.asarray(parent_sel, np.int64)
    child1_sel = np.asarray(child1_sel, np.int64)
    child2_sel = np.asarray(child2_sel, np.int64)

    clamp_rows = np.concatenate([
        parent_sel, NV + child1_sel, 2 * NV + child2_sel,
    ]).astype(np.int64)

    x0 = x.copy()
    x0[:, clamp_rows, 0:3] = inp[:, clamp_rows, :]

    deg = A.sum(axis=-1)
    deg_safe = np.where(deg == 0, np.float32(1.0), deg)
    d = np.where(deg == 0, np.float32(0.0),
                 deg_safe ** np.float32(-0.5)).astype(np.float32)
    A_norm = (A * d[:, None] * d[None, :]).astype(np.float32)

    with_bias = bool(np.any(b1) or np.any(b2) or np.any(b3) or np.any(b4))
    use_v4 = ((not with_bias) and _structure_matches(A_norm, d)
              and _d2_pattern_matches(d))

    trace = os.environ.get("KERNEL_TRACE", "") == "1"

    if use_v4:
        bf = ml_dtypes.bfloat16
        Xd = (d[None, :, None] * x0).astype(bf)                   # (B, N, 6)
        # xq[core, 32j+f, g*N + n] = Xd[core*8 + g*4 + j, n, f]
        Xr = Xd.reshape(NCORES, 2, 4, N, 6).transpose(0, 2, 4, 1, 3)
        xq = np.zeros((NCORES, 4, 32, 2 * N), bf)
        xq[:, :, 0:6, :] = Xr.reshape(NCORES, 4, 6, 2 * N)
        xq = xq.reshape(NCORES, 128, 2 * N)

        w1q = np.zeros((4, 32, 256), bf)
        w1q[:, 0:6, :] = W1.T.astype(bf)[None]
        w1q = np.ascontiguousarray(w1q.reshape(128, 256))
        w2pc = np.zeros((128, 288), bf)
        w2pc[:, 0:256] = np.ascontiguousarray(
            W2.T.reshape(2, 128, 128).transpose(1, 0, 2).reshape(128, 256)
        ).astype(bf)
        w2pc[:, 256:259] = (W3.T @ W4.T).astype(bf)

        nc = _build_program_v4()
        _split_multi_waits(nc)
        in_maps = [{"xq": xq[c], "w1q": w1q, "w2pc": w2pc}
                   for c in range(NCORES)]

        res = run_bass_kernel_spmd(nc, in_maps, list(range(NCORES)),
                                   trace=trace)
        LAST_RUN_INFO = {
            "exec_time_ns": res.exec_time_ns,
            "mean_exec_time_ns": res.mean_exec_time_ns,
            "max_exec_time_core_id": res.max_exec_time_core_id,
        }

        out = np.empty((B, N, 3), np.float32)
        for c in range(NCORES):
            o = np.asarray(res.results[c]["outq"], bf).astype(np.float32)
            # o[g, 32j+s, n] -> out[c*8 + g*4 + j, n, s]
            oi = o.reshape(2, 4, 32, N)[:, :, 0:3, :]     # (g, j, s, n)
            arr = oi.transpose(0, 1, 3, 2).reshape(IPC, N, 3)
            out[c * IPC:(c + 1) * IPC] = arr
        out *= d[None, :, None]
    else:
        AnT = np.ascontiguousarray(A_norm.T)
        A2T = np.ascontiguousarray((A_norm @ A_norm).T.astype(np.float32))
        W1T = np.ascontiguousarray(W1.T)
        W2Tp = np.ascontiguousarray(
            W2.T.reshape(2, 128, 128).transpose(1, 0, 2).reshape(128, 256))
        W34T = np.ascontiguousarray(W3.T @ W4.T)

        extra = {}
        if with_bias:
            s = A_norm.sum(axis=1).astype(np.float32)
            s2 = (A_norm @ s).astype(np.float32)
            p1t = np.einsum('f,n->fn', b1, s).astype(np.float32)
            p1t = p1t.reshape(2, 128, N).transpose(1, 0, 2).reshape(128, 2 * N)
            p2t = np.einsum('f,n->fn', b2, s).astype(np.float32)
            cp = (np.einsum('f,n->fn', W4 @ b3, s2) +
                  np.einsum('f,n->fn', b4, s)).astype(np.float32)
            cpt = np.tile(cp, (IPC, 1)).astype(np.float32)
            extra = {"p1t": np.ascontiguousarray(p1t),
                     "p2t": np.ascontiguousarray(p2t),
                     "cpt": np.ascontiguousarray(cpt)}

        xT_all = np.ascontiguousarray(
            x0.transpose(0, 2, 1).reshape(NCORES, IPC, 6, N))

        nc = _build_program_v1(with_bias)
        _split_multi_waits(nc)

        in_maps = []
        for c in range(NCORES):
            m = {
                "xT": xT_all[c], "anT": AnT, "a2T": A2T,
                "w1T": W1T, "w2Tp": W2Tp, "w34T": W34T,
            }
            m.update(extra)
            in_maps.append(m)

        res = run_bass_kernel_spmd(nc, in_maps, list(range(NCORES)),
                                   trace=trace)
        LAST_RUN_INFO = {
            "exec_time_ns": res.exec_time_ns,
            "mean_exec_time_ns": res.mean_exec_time_ns,
            "max_exec_time_core_id": res.max_exec_time_core_id,
        }

        out = np.empty((B, N, 3), np.float32)
        for c in range(NCORES):
            o = res.results[c]["outp"]
            for it in range(IPC):
                out[c * IPC + it] = o[it * 3:(it + 1) * 3, :].T
    out[:, clamp_rows, :] = inp[:, clamp_rows, :]
    return out


# revision 33
# speedup vs baseline: 1.2033x; 1.0380x over previous
"""Trainium2 Bass kernel for BatchedGNNModel (4-layer GCN over 3-rod chain graph).

Contract: kernel(**inputs) takes FULL unsharded inputs (as produced by
setup_inputs) and returns the FULL (64, 768, 3) float32 output.

Sharding: pure data parallel over batch — 8 items per NeuronCore on 8 cores,
identical SPMD program, weights replicated (marshaled on host).

v4 fast-path algorithm (zero biases, expected adjacency structure):
  A_norm = D·M·D with D = diag(d), d = deg^-1/2, M = tridiagonal-support ones
  + ~10 coefficient-1 sparse corrections. One application of M is
  S = tri_shift(U) + ents(U). d² is 1/3 everywhere except 8 columns
  ({0,255,511,767}: 1/2, {100,200,256,512}: 1/5), so every d²⊙ plane multiply
  is a tensor_scalar ×(1/3) plus 4 tiny strided column-fix multiplies — no
  d² plane in SBUF or DMA at all.

  Folded chain (relu is positively homogeneous, feature ops commute with
  node-diagonal scales):
    Gp = d²⊙(M (d⊙x))          [L1: DVE tri+ents+scale on packed x]
    h1 = relu(Gp @ W1ᵀ)        [f1: PE per-item 6-contract matmuls, ACT/DVE evac]
    u2 = h1 @ W2ᵀ              [f2: PE 128-contract matmuls per item pair]
    s2 = relu(M u2)            [agg2: DVE tri+ents+relu per pair]
    u3 = d²⊙(s2 @ WC)          [f4: PE item-packed 32-col matmuls, WC = W3ᵀW4ᵀ]
    out = d ⊙ M (d²⊙(M u3))    [tail per 4-item group: S3, m4, S4; d⊙ on host]

  Layout: items packed 4 per group at partition stride 32, features 0:6 of
  each 32-band; 2 groups as column blocks of 768. All activations bf16.
  Per-item software pipeline: f1(i)→f2(pair)→agg2(pair)→f4(i), with group 0's
  whole tail + output DMA issued mid-kernel so only group 1's tail is exposed.
  Input DMAs are row-sparse (only the 6 used partitions per band) and split
  across the two HWDGE queues (sync: x, scalar: weights) so descriptor
  writing parallelizes and first compute starts ~3us earlier.

Fallback path (nonzero biases or unexpected adjacency/d²): v1 dense program.

This image's walrus accepts only one sync-wait slot per instruction, so a
post-pass splits Tile's multi-wait instructions into single-wait NoOps.
"""

import os
import sys

import numpy as np

sys.path.insert(0, "/opt/trn_rl_repo")

import ml_dtypes
import concourse.bass as bass
import concourse.mybir as mybir
import concourse.tile as _tile_mod
from concourse.tile import TileContext
from concourse.vector_clock import ScopedClock
from concourse.bass_utils import run_bass_kernel_spmd


def _patched_drain_and_barrier(self, tick_clock, wait_clock):
    """The nix walrus in this image only supports one sync-wait slot on a
    Drain; Tile's kernel-tail drain carries one wait per ticked semaphore.
    Split the extra waits onto single-wait nops on the same (sync) engine —
    program order makes this equivalent before the all-engine barrier."""
    drain_inst = self.nc.sync.drain()
    wait_clock.add_sem_waits(
        drain_inst.ins, ScopedClock({None: tick_clock.global_clock}))
    waits = list(drain_inst.ins.sync_info.on_wait)
    if len(waits) > 1:
        import bass_rust
        drain_inst.ins.sync_info.on_wait = [waits[0]]
        for w in waits[1:]:
            nop = self.nc.sync.nop(nofuse=True)
            si = nop.ins.sync_info
            if si is None:
                nop.ins.sync_info = bass_rust.SyncInfo(on_wait=[w], on_update=[])
            else:
                si.on_wait = [w]
    self.nc.all_engine_barrier()
    assert self.sems is not None
    popped = self.nc._tile_sem_poison_stack.pop()
    assert popped is self._sem_poison
    self.nc.clear_and_free_semaphores(list(self.sems.allocated().values()))
    self.nc.all_engine_barrier()


_tile_mod.TileContext._drain_and_barrier = _patched_drain_and_barrier


def _split_multi_waits(nc):
    """This image's walrus supports a single sync-wait slot per instruction.
    Hoist all-but-one wait of any multi-wait instruction onto single-wait
    NoOps on the same engine, placed immediately before it (same per-engine
    program order => equivalent synchronization)."""
    for f in nc.m.functions:
        for bb in f.blocks:
            insts = list(bb.instructions)
            if not any(ins.sync_info and len(ins.sync_info.on_wait) > 1
                       for ins in insts):
                continue
            new = []
            for ins in insts:
                si = ins.sync_info
                if si is not None and len(si.on_wait) > 1:
                    waits = list(si.on_wait)
                    for w in waits[:-1]:
                        new.append(mybir.InstNoOp(
                            name=nc.get_next_instruction_name(),
                            sync_info=mybir.SyncInfo(on_wait=[w], on_update=[]),
                            bass_nofuse=True,
                            engine=ins.engine,
                        ))
                    si.on_wait = [waits[-1]]
                new.append(ins)
            bb.instructions = new


def _ensure_ntff_hook():
    """The agent image's antenv lacks axon_hooks; bass_utils imports it when
    trace=True. Install a shim and, if possible, the real ctypes profiler."""
    import types
    try:
        import antenv.axon_hooks  # noqa: F401
        return
    except Exception:
        pass
    try:
        import antenv
        mod = types.ModuleType("antenv.axon_hooks")
        state = {"h": None}
        mod.set_axon_ntff_profile_hook = lambda h: state.__setitem__("h", h)
        mod.get_axon_ntff_profile_hook = lambda: state["h"]
        sys.modules["antenv.axon_hooks"] = mod
        antenv.axon_hooks = mod
        try:
            from trn_agent_boot.trn_boot import _ntff_profile_via_ctypes
            mod.set_axon_ntff_profile_hook(
                _ntff_profile_via_ctypes("/opt/axon/libaxon_pjrt.so"))
        except Exception:
            pass
    except Exception:
        pass


_ensure_ntff_hook()

F32 = mybir.dt.float32
BF16 = mybir.dt.bfloat16
RELU = mybir.ActivationFunctionType.Relu
ADD = mybir.AluOpType.add
SUB = mybir.AluOpType.subtract
MULT = mybir.AluOpType.mult
MAX = mybir.AluOpType.max

B = 64
NV = 256
N = 3 * NV  # 768
NCORES = 8
IPC = B // NCORES  # 8 items per core

ONE_THIRD = float(np.float32(1.0) / np.float32(3.0))

LAST_RUN_INFO = {}

# Sparse corrections for one M application, coefficient-1 form, order-safe:
# (dst_col, 'S'|'U', src_col, op). S reads must precede writes to their col.
ENT_OPS = [
    (256, 'S', 100, ADD), (512, 'S', 200, ADD),
    (256, 'U', 255, SUB), (512, 'U', 511, SUB),
    (100, 'U', 256, ADD), (100, 'U', 257, ADD),
    (200, 'U', 512, ADD), (200, 'U', 513, ADD),
    (255, 'U', 256, SUB), (511, 'U', 512, SUB),
]

# d² = 1/3 everywhere except: ×3/2 at {0,255,511,767}, ×3/5 at {100,200,256,512}
FIX_GROUPS = [
    ((0, 1, 1), 1.5),
    ((255, 768, 256), 1.5),
    ((100, 201, 100), 0.6),
    ((256, 513, 256), 0.6),
]


def _np_tri_shift(U):
    S = U.copy()
    S[..., 1:, :] += U[..., :-1, :]
    S[..., :-1, :] += U[..., 1:, :]
    return S


def _np_ents(S, U):
    for (j, kind, k, op) in ENT_OPS:
        src = (S if kind == 'S' else U)[..., k, :].copy()
        if op is ADD:
            S[..., j, :] += src
        else:
            S[..., j, :] -= src
    return S


def _structure_matches(A_norm, d):
    """Does d ⊙ (tri+ents)(d ⊙ Z) reproduce A_norm @ Z?"""
    rng = np.random.default_rng(12345)
    Z = rng.standard_normal((1, N, 4)).astype(np.float32)
    want = np.einsum('ij,bjf->bif', A_norm, Z)
    U = d[None, :, None] * Z
    got = d[None, :, None] * _np_ents(_np_tri_shift(U), U)
    scale = np.abs(want).max() + 1e-30
    return np.abs(want - got).max() / scale < 1e-4


def _d2_pattern_matches(d):
    d2 = (d * d).astype(np.float32)
    e = np.full(N, ONE_THIRD, np.float32)
    for (start, stop, step), scale in FIX_GROUPS:
        e[start:stop:step] *= np.float32(scale)
    return np.allclose(d2, e, rtol=3e-5, atol=1e-7)


# ---------------------------------------------------------------------------
# v4 fast-path program
# ---------------------------------------------------------------------------

def _build_program_v4(warmup=16, dve_h1=(), debug=False):
    nc = bass.Bass()

    xq_d = nc.declare_dram_parameter("xq", [128, 2 * N], BF16, isOutput=False)
    w1q_d = nc.declare_dram_parameter("w1q", [128, 256], BF16, isOutput=False)
    w2pc_d = nc.declare_dram_parameter("w2pc", [128, 288], BF16, isOutput=False)
    outq_d = nc.declare_dram_parameter("outq", [2, 128, N], BF16, isOutput=True)
    if debug:
        dbg_d = {nm: nc.declare_dram_parameter(f"dbg_{nm}", shp, BF16,
                                               isOutput=True)
                 for nm, shp in [("gp", [128, 2 * N]), ("h1", [128, 2 * IPC * N]),
                                 ("u2", [128, IPC * N]), ("s2", [128, IPC * N]),
                                 ("u3", [128, 2 * N]), ("s3", [128, 2 * N]),
                                 ("m4", [128, 2 * N])]}

    def tri2(S, U, c0, eng=None):
        eng = eng or nc.vector
        eng.tensor_tensor(S[:, c0 + 1:c0 + N], U[:, c0 + 1:c0 + N],
                          U[:, c0:c0 + N - 1], op=ADD)
        eng.tensor_copy(S[:, c0:c0 + 1], U[:, c0:c0 + 1])
        eng.tensor_tensor(S[:, c0:c0 + N - 1], S[:, c0:c0 + N - 1],
                          U[:, c0 + 1:c0 + N], op=ADD)

    def ents2(S, U, c0, eng=None):
        eng = eng or nc.vector
        pairs = [
            (S[:, c0 + 256:c0 + 513:256], S[:, c0 + 100:c0 + 201:100], ADD),
            (S[:, c0 + 256:c0 + 513:256], U[:, c0 + 255:c0 + 512:256], SUB),
            (S[:, c0 + 100:c0 + 201:100], U[:, c0 + 256:c0 + 513:256], ADD),
            (S[:, c0 + 100:c0 + 201:100], U[:, c0 + 257:c0 + 514:256], ADD),
            (S[:, c0 + 255:c0 + 512:256], U[:, c0 + 256:c0 + 513:256], SUB),
        ]
        for dst, s_, op in pairs:
            eng.tensor_tensor(dst, dst, s_, op=op)

    def tri_pair(Sv, Uv, i0, i1):
        nc.vector.tensor_tensor(Sv[:, i0:i1, 1:N], Uv[:, i0:i1, 1:N],
                                Uv[:, i0:i1, 0:N - 1], op=ADD)
        nc.vector.tensor_copy(Sv[:, i0:i1, 0:1], Uv[:, i0:i1, 0:1])
        nc.vector.tensor_tensor(Sv[:, i0:i1, 0:N - 1], Sv[:, i0:i1, 0:N - 1],
                                Uv[:, i0:i1, 1:N], op=ADD)

    def ents_pair(Sv, Uv, i0, i1):
        I = slice(i0, i1)
        P = slice(0, 128)
        pairs = [
            (Sv[P, I, 256:513:256], Sv[P, I, 100:201:100], ADD),
            (Sv[P, I, 256:513:256], Uv[P, I, 255:512:256], SUB),
            (Sv[P, I, 100:201:100], Uv[P, I, 256:513:256], ADD),
            (Sv[P, I, 100:201:100], Uv[P, I, 257:514:256], ADD),
            (Sv[P, I, 255:512:256], Uv[P, I, 256:513:256], SUB),
        ]
        for dst, s_, op in pairs:
            nc.vector.tensor_tensor(dst, dst, s_, op=op)

    with TileContext(nc) as tc:
        with (
            tc.tile_pool(name="const", bufs=1) as cpool,
            tc.tile_pool(name="big", bufs=2, space="PSUM") as big,
            tc.tile_pool(name="p4", bufs=1, space="PSUM") as p4,
        ):
            # ---- constant tiles & DMAs ---------------------------------
            wz = cpool.tile([128, 512], BF16)
            nc.gpsimd.memset(wz[:, :], 0.0)
            # d² plane (one group width; identical for both groups), exact
            # values, built by the otherwise-idle gpsimd — no DMA.
            d2w = cpool.tile([128, N], BF16)
            nc.gpsimd.memset(d2w[:, :], ONE_THIRD)
            nc.gpsimd.memset(d2w[:, 0:1], 0.5)
            nc.gpsimd.memset(d2w[:, 255:768:256], 0.5)
            nc.gpsimd.memset(d2w[:, 100:201:100], 0.2)
            nc.gpsimd.memset(d2w[:, 256:513:256], 0.2)
            xpk = cpool.tile([128, 2 * N], BF16)
            nc.sync.dma_start(xpk[:, :], xq_d[:, :])
            w1 = cpool.tile([128, 256], BF16)
            nc.scalar.dma_start(w1[:, :], w1q_d[:, :])
            w2pc = cpool.tile([128, 288], BF16)
            nc.scalar.dma_start(w2pc[:, :], w2pc_d[:, :])

            G = cpool.tile([128, 2 * N], BF16)
            Gp = cpool.tile([128, 2 * N], BF16)
            h1 = cpool.tile([128, 2 * IPC * N], BF16)   # [kh*6144 + i*768 + n]
            u2 = cpool.tile([128, IPC * N], BF16)
            s2 = cpool.tile([128, IPC * N], BF16)
            U2v = u2[:, :].rearrange("p (i n) -> p i n", n=N)
            S2v = s2[:, :].rearrange("p (i n) -> p i n", n=N)
            u3g = [cpool.tile([128, N], BF16, name=f"u3g{g}") for g in range(2)]
            s3g = [cpool.tile([128, N], BF16, name=f"s3g{g}") for g in range(2)]
            m4g = [cpool.tile([128, N], BF16, name=f"m4g{g}") for g in range(2)]
            s4g = [cpool.tile([128, N], BF16, name=f"s4g{g}") for g in range(2)]
            wsink = cpool.tile([128, 1], F32)

            # ---- PE warm-up burst (no DMA dependency: zeros tile) ------
            # The PE reaches its 2.4GHz p-state only after 3us of GAP-FREE
            # execution and falls back to 1.2GHz after any idle gap, so
            # filler matmuls on the zeros tile are injected wherever the PE
            # would otherwise stall; they write PSUM regions that subsequent
            # start=True real matmuls reset.
            def fillers(T, k, w=512):
                for _ in range(k):
                    nc.tensor.matmul(T[:, 0:w], wz[:, 0:128], wz[:, 0:w],
                                     start=True, stop=True,
                                     skip_group_check=True)

            wps = p4.tile([128, N], F32, tag="p4")
            fillers(wps, warmup, w=256)

            # ---- pipeline stages ---------------------------------------
            C1 = 514

            def L1_group0_chunked():
                # columns 0:C1 first so f1_0's first (cs=0:512) matmul can
                # start before the rest of L1 finishes (all ent columns < C1)
                nc.vector.tensor_tensor(G[:, 1:C1], xpk[:, 1:C1],
                                        xpk[:, 0:C1 - 1], op=ADD)
                nc.vector.tensor_copy(G[:, 0:1], xpk[:, 0:1])
                nc.vector.tensor_tensor(G[:, 0:C1], G[:, 0:C1],
                                        xpk[:, 1:C1 + 1], op=ADD)
                ents2(G, xpk, 0)
                nc.vector.tensor_mul(Gp[:, 0:512], d2w[:, 0:512], G[:, 0:512])
                nc.vector.tensor_tensor(G[:, C1:N], xpk[:, C1:N],
                                        xpk[:, C1 - 1:N - 1], op=ADD)
                nc.vector.tensor_tensor(G[:, C1:N - 1], G[:, C1:N - 1],
                                        xpk[:, C1 + 1:N], op=ADD)
                nc.vector.tensor_mul(Gp[:, 512:N], d2w[:, 512:N],
                                     G[:, 512:N])

            def L1_group(g):
                c0 = g * N
                tri2(G, xpk, c0)
                ents2(G, xpk, c0)
                nc.vector.tensor_mul(Gp[:, c0:c0 + N], d2w[:, :],
                                     G[:, c0:c0 + N])

            def f1(i, fill=2):
                g, j = divmod(i, 4)
                T = big.tile([128, 2 * N], F32, tag="big", name=f"t1_{i}")
                fillers(T, fill)
                # psum chunks must not cross 512-f32 bank boundaries:
                # kh0 occupies tile cols 0:768 (chunks 512+256), kh1 cols
                # 768:1536 (chunks 256+512).
                chunks = (((0, 512), (512, 256)), ((0, 256), (256, 512)))
                for kh in range(2):
                    for cs, w in chunks[kh]:
                        nc.tensor.matmul(
                            T[:, kh * N + cs:kh * N + cs + w],
                            w1[32 * j:32 * j + 6, kh * 128:(kh + 1) * 128],
                            Gp[32 * j:32 * j + 6, g * N + cs:g * N + cs + w],
                            start=True, stop=True, tile_position=(32 * j, 0))
                return T

            def f1_evac(i, T):
                for kh in range(2):
                    dst = h1[:, kh * IPC * N + i * N:(kh * IPC + i + 1) * N]
                    src = T[:, kh * N:(kh + 1) * N]
                    if i in dve_h1 and kh == 1:
                        nc.vector.tensor_scalar(dst, src, 0.0, None, op0=MAX)
                    else:
                        nc.scalar.activation(dst, src, RELU)

            def f2(q, fill=4):
                i = 2 * q
                P = big.tile([128, 2 * N], F32, tag="big", name=f"t2_{q}")
                fillers(P, fill)
                for c in (0, 512, 1024):
                    for kh in range(2):
                        nc.tensor.matmul(
                            P[:, c:c + 512],
                            w2pc[:, kh * 128:(kh + 1) * 128],
                            h1[:, kh * IPC * N + i * N + c:
                               kh * IPC * N + i * N + c + 512],
                            start=(kh == 0), stop=(kh == 1))
                return P

            def u2_evac(q, P):
                i = 2 * q
                if q == 3:
                    # last pair is tail-critical: halve evac latency by
                    # splitting across ACT and DVE
                    nc.scalar.copy(u2[:, i * N:(i + 1) * N], P[:, 0:N])
                    nc.vector.tensor_copy(u2[:, (i + 1) * N:(i + 2) * N],
                                          P[:, N:2 * N])
                else:
                    nc.scalar.copy(u2[:, i * N:(i + 2) * N], P[:, 0:2 * N])

            def agg2(q):
                i = 2 * q
                tri_pair(S2v, U2v, i, i + 2)
                ents_pair(S2v, U2v, i, i + 2)
                sl = s2[:, i * N:(i + 2) * N]
                nc.vector.tensor_scalar(sl, sl, 0.0, None, op0=MAX)

            def f4(i, Q):
                g, j = divmod(i, 4)
                for cs, w in ((0, 512), (512, 256)):
                    nc.tensor.matmul(
                        Q[32 * j:32 * j + 32, cs:cs + w],
                        w2pc[:, 256:288],
                        s2[:, i * N + cs:i * N + cs + w],
                        start=True, stop=True, tile_position=(0, 32 * j))

            def u3_evac(g, Q):
                nc.vector.tensor_mul(u3g[g][:, :], d2w[:, :], Q[:, 0:N])

            def tail_S3(g):
                tri2(s3g[g], u3g[g], 0)
                ents2(s3g[g], u3g[g], 0)

            def tail_m4(g):
                nc.vector.tensor_mul(m4g[g][:, :], d2w[:, :], s3g[g][:, :])

            def tail_S4(g):
                tri2(s4g[g], m4g[g], 0)
                ents2(s4g[g], m4g[g], 0)

            def out_dma(g):
                nc.sync.dma_start(outq_d[g], s4g[g][:, :])

            # ---- schedule ----------------------------------------------
            # ACT stream keeps each pair's u2 evac AFTER the next pair's
            # first f1 evacs so f2(q+1) is never blocked behind u2e(q);
            # group-0 tail ops are slotted into DVE's between-pair gaps.
            L1_group0_chunked()
            T0 = f1(0)
            f1_evac(0, T0)
            T1 = f1(1)
            L1_group(1)
            nc.vector.tensor_copy(wsink[:, :], wps[:, 0:1])
            f1_evac(1, T1)
            P0 = f2(0)
            T2 = f1(2)
            f1_evac(2, T2)
            u2_evac(0, P0)
            T3 = f1(3)
            f1_evac(3, T3)
            agg2(0)
            P1 = f2(1)
            Qg0 = p4.tile([128, N], F32, tag="p4", name="q0")
            T4 = f1(4)
            f1_evac(4, T4)
            u2_evac(1, P1)
            f4(0, Qg0)
            f4(1, Qg0)
            T5 = f1(5)
            f1_evac(5, T5)
            agg2(1)
            f4(2, Qg0)
            f4(3, Qg0)
            u3_evac(0, Qg0)
            P2 = f2(2)
            T6 = f1(6)
            f1_evac(6, T6)
            u2_evac(2, P2)
            tail_S3(0)
            tail_m4(0)
            T7 = f1(7)
            f1_evac(7, T7)
            agg2(2)
            tail_S4(0)
            out_dma(0)
            P3 = f2(3)
            u2_evac(3, P3)
            agg2(3)
            Qg1 = p4.tile([128, N], F32, tag="p4", name="q1")
            fillers(Qg1, 8)
            for i in range(4, 8):
                f4(i, Qg1)
            u3_evac(1, Qg1)
            tail_S3(1)
            tail_m4(1)
            tail_S4(1)
            out_dma(1)

            if debug:
                nc.sync.dma_start(dbg_d["gp"][:, :], Gp[:, :])
                nc.sync.dma_start(dbg_d["h1"][:, :], h1[:, :])
                nc.sync.dma_start(dbg_d["u2"][:, :], u2[:, :])
                nc.sync.dma_start(dbg_d["s2"][:, :], s2[:, :])
                for g in range(2):
                    nc.sync.dma_start(dbg_d["u3"][:, g * N:(g + 1) * N],
                                      u3g[g][:, :])
                    nc.sync.dma_start(dbg_d["s3"][:, g * N:(g + 1) * N],
                                      s3g[g][:, :])
                    nc.sync.dma_start(dbg_d["m4"][:, g * N:(g + 1) * N],
                                      m4g[g][:, :])

    return nc


# ---------------------------------------------------------------------------
# v1 dense fallback (bias / unexpected adjacency)
# ---------------------------------------------------------------------------

def _build_program_v1(with_bias: bool):
    nc = bass.Bass()
    KT = N // 128

    xT_d = nc.declare_dram_parameter("xT", [IPC, 6, N], F32, isOutput=False)
    anT_d = nc.declare_dram_parameter("anT", [N, N], F32, isOutput=False)
    a2T_d = nc.declare_dram_parameter("a2T", [N, N], F32, isOutput=False)
    w1T_d = nc.declare_dram_parameter("w1T", [6, 256], F32, isOutput=False)
    w2Tp_d = nc.declare_dram_parameter("w2Tp", [128, 256], F32, isOutput=False)
    w34T_d = nc.declare_dram_parameter("w34T", [128, 3], F32, isOutput=False)
    if with_bias:
        p1t_d = nc.declare_dram_parameter("p1t", [128, 2 * N], F32, isOutput=False)
        p2t_d = nc.declare_dram_parameter("p2t", [128, N], F32, isOutput=False)
        cpt_d = nc.declare_dram_parameter("cpt", [3 * IPC, N], F32, isOutput=False)
    out_d = nc.declare_dram_parameter("outp", [3 * IPC, N], F32, isOutput=True)

    with TileContext(nc) as tc:
        with (
            tc.tile_pool(name="const", bufs=1) as cpool,
            tc.tile_pool(name="acts", bufs=2) as apool,
            tc.tile_pool(name="psf", bufs=2, space="PSUM") as psf,
            tc.tile_pool(name="psa", bufs=3, space="PSUM") as psa,
        ):
            anT = cpool.tile([128, KT * N], F32)
            nc.sync.dma_start(
                anT[:, :].rearrange("p (k j) -> p k j", j=N),
                anT_d[:, :].rearrange("(k p) j -> p k j", p=128))
            a2T = cpool.tile([128, KT * N], F32)
            nc.sync.dma_start(
                a2T[:, :].rearrange("p (k j) -> p k j", j=N),
                a2T_d[:, :].rearrange("(k p) j -> p k j", p=128))
            w1T = cpool.tile([6, 256], F32)
            nc.sync.dma_start(w1T[:, :], w1T_d[:, :])
            w2Tp = cpool.tile([128, 256], F32)
            nc.sync.dma_start(w2Tp[:, :], w2Tp_d[:, :])
            w34T = cpool.tile([128, 3], F32)
            nc.sync.dma_start(w34T[:, :], w34T_d[:, :])
            if with_bias:
                p1t = cpool.tile([128, 2 * N], F32)
                nc.sync.dma_start(p1t[:, :], p1t_d[:, :])
                p2t = cpool.tile([128, N], F32)
                nc.sync.dma_start(p2t[:, :], p2t_d[:, :])
                cpt = cpool.tile([3 * IPC, N], F32)
                nc.sync.dma_start(cpt[:, :], cpt_d[:, :])

            z34 = cpool.tile([128, KT * 3 * IPC], F32)

            for it in range(IPC):
                xT = apool.tile([6, N], F32, tag="xT")
                nc.sync.dma_start(xT[:, :], xT_d[it])

                z1 = apool.tile([128, KT * 256], F32, tag="z1")
                for m in range(KT):
                    ps = psf.tile([128, 256], F32, tag="feat")
                    nc.tensor.matmul(
                        ps[:, :], xT[:, m * 128:(m + 1) * 128], w1T[:, :],
                        start=True, stop=True,
                    )
                    nc.vector.tensor_copy(z1[:, m * 256:(m + 1) * 256], ps[:, :])

                h1t = apool.tile([128, 2 * N], F32, tag="h1t")
                for fh in range(2):
                    for ns in range(2):
                        ps = psa.tile([128, 384], F32, tag="agg")
                        for k in range(KT):
                            nc.tensor.matmul(
                                ps[:, :],
                                z1[:, k * 256 + fh * 128: k * 256 + fh * 128 + 128],
                                anT[:, k * N + ns * 384: k * N + ns * 384 + 384],
                                start=(k == 0), stop=(k == KT - 1),
                            )
                        dst = h1t[:, fh * N + ns * 384: fh * N + ns * 384 + 384]
                        if with_bias:
                            nc.vector.tensor_tensor(
                                dst, ps[:, :],
                                p1t[:, fh * N + ns * 384: fh * N + ns * 384 + 384],
                                op=ADD,
                            )
                            nc.scalar.activation(dst, dst, RELU)
                        else:
                            nc.scalar.activation(dst, ps[:, :], RELU)

                z2 = apool.tile([128, KT * 128], F32, tag="z2")
                for m in range(KT):
                    ps = psf.tile([128, 128], F32, tag="feat")
                    for kh in range(2):
                        nc.tensor.matmul(
                            ps[:, :],
                            h1t[:, kh * N + m * 128: kh * N + m * 128 + 128],
                            w2Tp[:, kh * 128:(kh + 1) * 128],
                            start=(kh == 0), stop=(kh == 1),
                        )
                    nc.vector.tensor_copy(z2[:, m * 128:(m + 1) * 128], ps[:, :])

                h2t = apool.tile([128, N], F32, tag="h2t")
                for ns in range(2):
                    ps = psa.tile([128, 384], F32, tag="agg")
                    for k in range(KT):
                        nc.tensor.matmul(
                            ps[:, :],
                            z2[:, k * 128:(k + 1) * 128],
                            anT[:, k * N + ns * 384: k * N + ns * 384 + 384],
                            start=(k == 0), stop=(k == KT - 1),
                        )
                    dst = h2t[:, ns * 384: ns * 384 + 384]
                    if with_bias:
                        nc.vector.tensor_tensor(
                            dst, ps[:, :], p2t[:, ns * 384: ns * 384 + 384],
                            op=ADD,
                        )
                        nc.scalar.activation(dst, dst, RELU)
                    else:
                        nc.scalar.activation(dst, ps[:, :], RELU)

                for m in range(KT):
                    ps = psf.tile([128, 3], F32, tag="feat")
                    nc.tensor.matmul(
                        ps[:, :], h2t[:, m * 128:(m + 1) * 128], w34T[:, :],
                        start=True, stop=True,
                    )
                    base = m * 3 * IPC + it * 3
                    nc.vector.tensor_copy(z34[:, base: base + 3], ps[:, :])

            outT = cpool.tile([3 * IPC, N], F32)
            for ns in range(2):
                ps = psa.tile([3 * IPC, 384], F32, tag="agg")
                for k in range(KT):
                    nc.tensor.matmul(
                        ps[:, :],
                        z34[:, k * 3 * IPC:(k + 1) * 3 * IPC],
                        a2T[:, k * N + ns * 384: k * N + ns * 384 + 384],
                        start=(k == 0), stop=(k == KT - 1),
                    )
                dst = outT[:, ns * 384: ns * 384 + 384]
                if with_bias:
                    nc.vector.tensor_tensor(
                        dst, ps[:, :], cpt[:, ns * 384: ns * 384 + 384],
                        op=ADD,
                    )
                else:
                    nc.vector.tensor_copy(dst, ps[:, :])
            nc.sync.dma_start(out_d[:, :], outT[:, :])

    return nc


def kernel(x, inputs, adjacency, W1, b1, W2, b2, W3, b3, W4, b4,
           parent_sel, child1_sel, child2_sel):
    global LAST_RUN_INFO
    x = np.asarray(x, np.float32)
    inp = np.asarray(inputs, np.float32)
    A = np.asarray(adjacency, np.float32)
    W1 = np.asarray(W1, np.float32); b1 = np.asarray(b1, np.float32)
    W2 = np.asarray(W2, np.float32); b2 = np.asarray(b2, np.float32)
    W3 = np.asarray(W3, np.float32); b3 = np.asarray(b3, np.float32)
    W4 = np.asarray(W4, np.float32); b4 = np.asarray(b4, np.float32)
    parent_sel = np.asarray(parent_sel, np.int64)
    child1_sel = np.asarray(child1_sel, np.int64)
    child2_sel = np.asarray(child2_sel, np.int64)

    clamp_rows = np.concatenate([
        parent_sel, NV + child1_sel, 2 * NV + child2_sel,
    ]).astype(np.int64)

    x0 = x.copy()
    x0[:, clamp_rows, 0:3] = inp[:, clamp_rows, :]

    deg = A.sum(axis=-1)
    deg_safe = np.where(deg == 0, np.float32(1.0), deg)
    d = np.where(deg == 0, np.float32(0.0),
                 deg_safe ** np.float32(-0.5)).astype(np.float32)
    A_norm = (A * d[:, None] * d[None, :]).astype(np.float32)

    with_bias = bool(np.any(b1) or np.any(b2) or np.any(b3) or np.any(b4))
    use_v4 = ((not with_bias) and _structure_matches(A_norm, d)
              and _d2_pattern_matches(d))

    trace = os.environ.get("KERNEL_TRACE", "") == "1"

    if use_v4:
        bf = ml_dtypes.bfloat16
        Xd = (d[None, :, None] * x0).astype(bf)                   # (B, N, 6)
        # xq[core, 32j+f, g*N + n] = Xd[core*8 + g*4 + j, n, f]
        Xr = Xd.reshape(NCORES, 2, 4, N, 6).transpose(0, 2, 4, 1, 3)
        xq = np.zeros((NCORES, 4, 32, 2 * N), bf)
        xq[:, :, 0:6, :] = Xr.reshape(NCORES, 4, 6, 2 * N)
        xq = xq.reshape(NCORES, 128, 2 * N)

        w1q = np.zeros((4, 32, 256), bf)
        w1q[:, 0:6, :] = W1.T.astype(bf)[None]
        w1q = np.ascontiguousarray(w1q.reshape(128, 256))
        w2pc = np.zeros((128, 288), bf)
        w2pc[:, 0:256] = np.ascontiguousarray(
            W2.T.reshape(2, 128, 128).transpose(1, 0, 2).reshape(128, 256)
        ).astype(bf)
        w2pc[:, 256:259] = (W3.T @ W4.T).astype(bf)

        nc = _build_program_v4()
        _split_multi_waits(nc)
        in_maps = [{"xq": xq[c], "w1q": w1q, "w2pc": w2pc}
                   for c in range(NCORES)]

        res = run_bass_kernel_spmd(nc, in_maps, list(range(NCORES)),
                                   trace=trace)
        LAST_RUN_INFO = {
            "exec_time_ns": res.exec_time_ns,
            "mean_exec_time_ns": res.mean_exec_time_ns,
            "max_exec_time_core_id": res.max_exec_time_core_id,
        }

        out = np.empty((B, N, 3), np.float32)
        for c in range(NCORES):
            o = np.asarray(res.results[c]["outq"], bf).astype(np.float32)
            # o[g, 32j+s, n] -> out[c*8 + g*4 + j, n, s]
            oi = o.reshape(2, 4, 32, N)[:, :, 0:3, :]     # (g, j, s, n)
            arr = oi.transpose(0, 1, 3, 2).reshape(IPC, N, 3)
            out[c * IPC:(c + 1) * IPC] = arr
        out *= d[None, :, None]
    else:
        AnT = np.ascontiguousarray(A_norm.T)
        A2T = np.ascontiguousarray((A_norm @ A_norm).T.astype(np.float32))
        W1T = np.ascontiguousarray(W1.T)
        W2Tp = np.ascontiguousarray(
            W2.T.reshape(2, 128, 128).transpose(1, 0, 2).reshape(128, 256))
        W34T = np.ascontiguousarray(W3.T @ W4.T)

        extra = {}
        if with_bias:
            s = A_norm.sum(axis=1).astype(np.float32)
            s2 = (A_norm @ s).astype(np.float32)
            p1t = np.einsum('f,n->fn', b1, s).astype(np.float32)
            p1t = p1t.reshape(2, 128, N).transpose(1, 0, 2).reshape(128, 2 * N)
            p2t = np.einsum('f,n->fn', b2, s).astype(np.float32)
            cp = (np.einsum('f,n->fn', W4 @ b3, s2) +
                  np.einsum('f,n->fn', b4, s)).astype(np.float32)
            cpt = np.tile(cp, (IPC, 1)).astype(np.float32)
            extra = {"p1t": np.ascontiguousarray(p1t),
                     "p2t": np.ascontiguousarray(p2t),
                     "cpt": np.ascontiguousarray(cpt)}

        xT_all = np.ascontiguousarray(
            x0.transpose(0, 2, 1).reshape(NCORES, IPC, 6, N))

        nc = _build_program_v1(with_bias)
        _split_multi_waits(nc)

        in_maps = []
        for c in range(NCORES):
            m = {
                "xT": xT_all[c], "anT": AnT, "a2T": A2T,
                "w1T": W1T, "w2Tp": W2Tp, "w34T": W34T,
            }
            m.update(extra)
            in_maps.append(m)

        res = run_bass_kernel_spmd(nc, in_maps, list(range(NCORES)),
                                   trace=trace)
        LAST_RUN_INFO = {
            "exec_time_ns": res.exec_time_ns,
            "mean_exec_time_ns": res.mean_exec_time_ns,
            "max_exec_time_core_id": res.max_exec_time_core_id,
        }

        out = np.empty((B, N, 3), np.float32)
        for c in range(NCORES):
            o = res.results[c]["outp"]
            for it in range(IPC):
                out[c * IPC + it] = o[it * 3:(it + 1) * 3, :].T
    out[:, clamp_rows, :] = inp[:, clamp_rows, :]
    return out


# revision 38
# speedup vs baseline: 1.2161x; 1.0107x over previous
"""Trainium2 Bass kernel for BatchedGNNModel (4-layer GCN over 3-rod chain graph).

Contract: kernel(**inputs) takes FULL unsharded inputs (as produced by
setup_inputs) and returns the FULL (64, 768, 3) float32 output.

Sharding: pure data parallel over batch — 8 items per NeuronCore on 8 cores,
identical SPMD program, weights replicated (marshaled on host).

v4 fast-path algorithm (zero biases, expected adjacency structure):
  A_norm = D·M·D with D = diag(d), d = deg^-1/2, M = tridiagonal-support ones
  + ~10 coefficient-1 sparse corrections. One application of M is
  S = tri_shift(U) + ents(U). d² is 1/3 everywhere except 8 columns
  ({0,255,511,767}: 1/2, {100,200,256,512}: 1/5), so every d²⊙ plane multiply
  is a tensor_scalar ×(1/3) plus 4 tiny strided column-fix multiplies — no
  d² plane in SBUF or DMA at all.

  Folded chain (relu is positively homogeneous, feature ops commute with
  node-diagonal scales):
    Gp = d²⊙(M (d⊙x))          [L1: DVE tri+ents+scale on packed x]
    h1 = relu(Gp @ W1ᵀ)        [f1: PE per-item 6-contract matmuls, ACT/DVE evac]
    u2 = h1 @ W2ᵀ              [f2: PE 128-contract matmuls per item pair]
    s2 = relu(M u2)            [agg2: DVE tri+ents+relu per pair]
    u3 = d²⊙(s2 @ WC)          [f4: PE item-packed 32-col matmuls, WC = W3ᵀW4ᵀ]
    out = d ⊙ M (d²⊙(M u3))    [tail per 4-item group: S3, m4, S4; d⊙ on host]

  Layout: items packed 4 per group at partition stride 32, features 0:6 of
  each 32-band; 2 groups as column blocks of 768. All activations bf16.
  Per-item software pipeline: f1(i)→f2(pair)→agg2(pair)→f4(i), with group 0's
  whole tail + output DMA issued mid-kernel so only group 1's tail is exposed.
  Input DMAs are row-sparse (only the 6 used partitions per band) and split
  across the two HWDGE queues (sync: x, scalar: weights) so descriptor
  writing parallelizes and first compute starts ~3us earlier.

Fallback path (nonzero biases or unexpected adjacency/d²): v1 dense program.

This image's walrus accepts only one sync-wait slot per instruction, so a
post-pass splits Tile's multi-wait instructions into single-wait NoOps.
"""

import os
import sys

import numpy as np

sys.path.insert(0, "/opt/trn_rl_repo")

import ml_dtypes
import concourse.bass as bass
import concourse.mybir as mybir
import concourse.tile as _tile_mod
from concourse.tile import TileContext
from concourse.vector_clock import ScopedClock
from concourse.bass_utils import run_bass_kernel_spmd


def _patched_drain_and_barrier(self, tick_clock, wait_clock):
    """The nix walrus in this image only supports one sync-wait slot on a
    Drain; Tile's kernel-tail drain carries one wait per ticked semaphore.
    Split the extra waits onto single-wait nops on the same (sync) engine —
    program order makes this equivalent before the all-engine barrier."""
    drain_inst = self.nc.sync.drain()
    wait_clock.add_sem_waits(
        drain_inst.ins, ScopedClock({None: tick_clock.global_clock}))
    waits = list(drain_inst.ins.sync_info.on_wait)
    if len(waits) > 1:
        import bass_rust
        drain_inst.ins.sync_info.on_wait = [waits[0]]
        for w in waits[1:]:
            nop = self.nc.sync.nop(nofuse=True)
            si = nop.ins.sync_info
            if si is None:
                nop.ins.sync_info = bass_rust.SyncInfo(on_wait=[w], on_update=[])
            else:
                si.on_wait = [w]
    self.nc.all_engine_barrier()
    assert self.sems is not None
    popped = self.nc._tile_sem_poison_stack.pop()
    assert popped is self._sem_poison
    self.nc.clear_and_free_semaphores(list(self.sems.allocated().values()))
    self.nc.all_engine_barrier()


_tile_mod.TileContext._drain_and_barrier = _patched_drain_and_barrier


def _split_multi_waits(nc):
    """This image's walrus supports a single sync-wait slot per instruction.
    Hoist all-but-one wait of any multi-wait instruction onto single-wait
    NoOps on the same engine, placed immediately before it (same per-engine
    program order => equivalent synchronization)."""
    for f in nc.m.functions:
        for bb in f.blocks:
            insts = list(bb.instructions)
            if not any(ins.sync_info and len(ins.sync_info.on_wait) > 1
                       for ins in insts):
                continue
            new = []
            for ins in insts:
                si = ins.sync_info
                if si is not None and len(si.on_wait) > 1:
                    waits = list(si.on_wait)
                    for w in waits[:-1]:
                        new.append(mybir.InstNoOp(
                            name=nc.get_next_instruction_name(),
                            sync_info=mybir.SyncInfo(on_wait=[w], on_update=[]),
                            bass_nofuse=True,
                            engine=ins.engine,
                        ))
                    si.on_wait = [waits[-1]]
                new.append(ins)
            bb.instructions = new


def _ensure_ntff_hook():
    """The agent image's antenv lacks axon_hooks; bass_utils imports it when
    trace=True. Install a shim and, if possible, the real ctypes profiler."""
    import types
    try:
        import antenv.axon_hooks  # noqa: F401
        return
    except Exception:
        pass
    try:
        import antenv
        mod = types.ModuleType("antenv.axon_hooks")
        state = {"h": None}
        mod.set_axon_ntff_profile_hook = lambda h: state.__setitem__("h", h)
        mod.get_axon_ntff_profile_hook = lambda: state["h"]
        sys.modules["antenv.axon_hooks"] = mod
        antenv.axon_hooks = mod
        try:
            from trn_agent_boot.trn_boot import _ntff_profile_via_ctypes
            mod.set_axon_ntff_profile_hook(
                _ntff_profile_via_ctypes("/opt/axon/libaxon_pjrt.so"))
        except Exception:
            pass
    except Exception:
        pass


_ensure_ntff_hook()

F32 = mybir.dt.float32
BF16 = mybir.dt.bfloat16
RELU = mybir.ActivationFunctionType.Relu
ADD = mybir.AluOpType.add
SUB = mybir.AluOpType.subtract
MULT = mybir.AluOpType.mult
MAX = mybir.AluOpType.max

B = 64
NV = 256
N = 3 * NV  # 768
NCORES = 8
IPC = B // NCORES  # 8 items per core

ONE_THIRD = float(np.float32(1.0) / np.float32(3.0))

LAST_RUN_INFO = {}

# Sparse corrections for one M application, coefficient-1 form, order-safe:
# (dst_col, 'S'|'U', src_col, op). S reads must precede writes to their col.
ENT_OPS = [
    (256, 'S', 100, ADD), (512, 'S', 200, ADD),
    (256, 'U', 255, SUB), (512, 'U', 511, SUB),
    (100, 'U', 256, ADD), (100, 'U', 257, ADD),
    (200, 'U', 512, ADD), (200, 'U', 513, ADD),
    (255, 'U', 256, SUB), (511, 'U', 512, SUB),
]

# d² = 1/3 everywhere except: ×3/2 at {0,255,511,767}, ×3/5 at {100,200,256,512}
FIX_GROUPS = [
    ((0, 1, 1), 1.5),
    ((255, 768, 256), 1.5),
    ((100, 201, 100), 0.6),
    ((256, 513, 256), 0.6),
]


def _np_tri_shift(U):
    S = U.copy()
    S[..., 1:, :] += U[..., :-1, :]
    S[..., :-1, :] += U[..., 1:, :]
    return S


def _np_ents(S, U):
    for (j, kind, k, op) in ENT_OPS:
        src = (S if kind == 'S' else U)[..., k, :].copy()
        if op is ADD:
            S[..., j, :] += src
        else:
            S[..., j, :] -= src
    return S


def _structure_matches(A_norm, d):
    """Does d ⊙ (tri+ents)(d ⊙ Z) reproduce A_norm @ Z?"""
    rng = np.random.default_rng(12345)
    Z = rng.standard_normal((1, N, 4)).astype(np.float32)
    want = np.einsum('ij,bjf->bif', A_norm, Z)
    U = d[None, :, None] * Z
    got = d[None, :, None] * _np_ents(_np_tri_shift(U), U)
    scale = np.abs(want).max() + 1e-30
    return np.abs(want - got).max() / scale < 1e-4


def _d2_pattern_matches(d):
    d2 = (d * d).astype(np.float32)
    e = np.full(N, ONE_THIRD, np.float32)
    for (start, stop, step), scale in FIX_GROUPS:
        e[start:stop:step] *= np.float32(scale)
    return np.allclose(d2, e, rtol=3e-5, atol=1e-7)


# ---------------------------------------------------------------------------
# v4 fast-path program
# ---------------------------------------------------------------------------

def _build_program_v4(warmup=44, dve_h1=(0, 1), debug=False):
    nc = bass.Bass()

    xq_d = nc.declare_dram_parameter("xq", [128, 2 * N], BF16, isOutput=False)
    w1q_d = nc.declare_dram_parameter("w1q", [128, 256], BF16, isOutput=False)
    w2pc_d = nc.declare_dram_parameter("w2pc", [128, 288], BF16, isOutput=False)
    outq_d = nc.declare_dram_parameter("outq", [2, 128, N], BF16, isOutput=True)
    if debug:
        dbg_d = {nm: nc.declare_dram_parameter(f"dbg_{nm}", shp, BF16,
                                               isOutput=True)
                 for nm, shp in [("gp", [128, 2 * N]), ("h1", [128, 2 * IPC * N]),
                                 ("u2", [128, IPC * N]), ("s2", [128, IPC * N]),
                                 ("u3", [128, 2 * N]), ("s3", [128, 2 * N]),
                                 ("m4", [128, 2 * N])]}

    def tri2(S, U, c0, eng=None):
        eng = eng or nc.vector
        eng.tensor_tensor(S[:, c0 + 1:c0 + N], U[:, c0 + 1:c0 + N],
                          U[:, c0:c0 + N - 1], op=ADD)
        eng.tensor_copy(S[:, c0:c0 + 1], U[:, c0:c0 + 1])
        eng.tensor_tensor(S[:, c0:c0 + N - 1], S[:, c0:c0 + N - 1],
                          U[:, c0 + 1:c0 + N], op=ADD)

    def ents2(S, U, c0, eng=None):
        eng = eng or nc.vector
        pairs = [
            (S[:, c0 + 256:c0 + 513:256], S[:, c0 + 100:c0 + 201:100], ADD),
            (S[:, c0 + 256:c0 + 513:256], U[:, c0 + 255:c0 + 512:256], SUB),
            (S[:, c0 + 100:c0 + 201:100], U[:, c0 + 256:c0 + 513:256], ADD),
            (S[:, c0 + 100:c0 + 201:100], U[:, c0 + 257:c0 + 514:256], ADD),
            (S[:, c0 + 255:c0 + 512:256], U[:, c0 + 256:c0 + 513:256], SUB),
        ]
        for dst, s_, op in pairs:
            eng.tensor_tensor(dst, dst, s_, op=op)

    def tri_pair(Sv, Uv, i0, i1):
        nc.vector.tensor_tensor(Sv[:, i0:i1, 1:N], Uv[:, i0:i1, 1:N],
                                Uv[:, i0:i1, 0:N - 1], op=ADD)
        nc.vector.tensor_copy(Sv[:, i0:i1, 0:1], Uv[:, i0:i1, 0:1])
        nc.vector.tensor_tensor(Sv[:, i0:i1, 0:N - 1], Sv[:, i0:i1, 0:N - 1],
                                Uv[:, i0:i1, 1:N], op=ADD)

    def ents_pair(Sv, Uv, i0, i1):
        I = slice(i0, i1)
        P = slice(0, 128)
        pairs = [
            (Sv[P, I, 256:513:256], Sv[P, I, 100:201:100], ADD),
            (Sv[P, I, 256:513:256], Uv[P, I, 255:512:256], SUB),
            (Sv[P, I, 100:201:100], Uv[P, I, 256:513:256], ADD),
            (Sv[P, I, 100:201:100], Uv[P, I, 257:514:256], ADD),
            (Sv[P, I, 255:512:256], Uv[P, I, 256:513:256], SUB),
        ]
        for dst, s_, op in pairs:
            nc.vector.tensor_tensor(dst, dst, s_, op=op)

    with TileContext(nc) as tc:
        with (
            tc.tile_pool(name="const", bufs=1) as cpool,
            tc.tile_pool(name="big", bufs=2, space="PSUM") as big,
            tc.tile_pool(name="p4", bufs=1, space="PSUM") as p4,
        ):
            # ---- constant tiles & DMAs ---------------------------------
            wz = cpool.tile([128, 512], BF16)
            nc.gpsimd.memset(wz[:, :], 0.0)
            # d² plane (one group width; identical for both groups), exact
            # values, built by the otherwise-idle gpsimd — no DMA.
            d2w = cpool.tile([128, N], BF16)
            nc.gpsimd.memset(d2w[:, :], ONE_THIRD)
            nc.gpsimd.memset(d2w[:, 0:1], 0.5)
            nc.gpsimd.memset(d2w[:, 255:768:256], 0.5)
            nc.gpsimd.memset(d2w[:, 100:201:100], 0.2)
            nc.gpsimd.memset(d2w[:, 256:513:256], 0.2)
            xpk = cpool.tile([128, 2 * N], BF16)
            nc.sync.dma_start(xpk[:, :], xq_d[:, :])
            w1 = cpool.tile([128, 256], BF16)
            nc.scalar.dma_start(w1[:, :], w1q_d[:, :])
            w2pc = cpool.tile([128, 288], BF16)
            nc.scalar.dma_start(w2pc[:, :], w2pc_d[:, :])

            G = cpool.tile([128, 2 * N], BF16)
            Gp = cpool.tile([128, 2 * N], BF16)
            h1 = cpool.tile([128, 2 * IPC * N], BF16)   # [kh*6144 + i*768 + n]
            u2 = cpool.tile([128, IPC * N], BF16)
            s2 = cpool.tile([128, IPC * N], BF16)
            U2v = u2[:, :].rearrange("p (i n) -> p i n", n=N)
            S2v = s2[:, :].rearrange("p (i n) -> p i n", n=N)
            u3g = [cpool.tile([128, N], BF16, name=f"u3g{g}") for g in range(2)]
            s3g = [cpool.tile([128, N], BF16, name=f"s3g{g}") for g in range(2)]
            m4g = [cpool.tile([128, N], BF16, name=f"m4g{g}") for g in range(2)]
            s4g = [cpool.tile([128, N], BF16, name=f"s4g{g}") for g in range(2)]
            wsink = cpool.tile([128, 1], F32)

            # ---- PE warm-up burst (no DMA dependency: zeros tile) ------
            # The PE reaches its 2.4GHz p-state only after 3us of GAP-FREE
            # execution and falls back to 1.2GHz after any idle gap, so
            # filler matmuls on the zeros tile are injected wherever the PE
            # would otherwise stall; they write PSUM regions that subsequent
            # start=True real matmuls reset.
            def fillers(T, k, w=512, c0=0):
                for _ in range(k):
                    nc.tensor.matmul(T[:, c0:c0 + w], wz[:, 0:128],
                                     wz[:, 0:w], start=True, stop=True,
                                     skip_group_check=True)

            wps = p4.tile([128, N], F32, tag="p4")
            fillers(wps, warmup, w=256)

            # ---- pipeline stages ---------------------------------------
            C1 = 514

            def L1_group0_chunked():
                # columns 0:C1 first so f1_0's first (cs=0:512) matmul can
                # start before the rest of L1 finishes (all ent columns < C1)
                nc.vector.tensor_tensor(G[:, 1:C1], xpk[:, 1:C1],
                                        xpk[:, 0:C1 - 1], op=ADD)
                nc.vector.tensor_copy(G[:, 0:1], xpk[:, 0:1])
                nc.vector.tensor_tensor(G[:, 0:C1], G[:, 0:C1],
                                        xpk[:, 1:C1 + 1], op=ADD)
                ents2(G, xpk, 0)
                nc.vector.tensor_mul(Gp[:, 0:512], d2w[:, 0:512], G[:, 0:512])
                nc.vector.tensor_tensor(G[:, C1:N], xpk[:, C1:N],
                                        xpk[:, C1 - 1:N - 1], op=ADD)
                nc.vector.tensor_tensor(G[:, C1:N - 1], G[:, C1:N - 1],
                                        xpk[:, C1 + 1:N], op=ADD)
                nc.vector.tensor_mul(Gp[:, 512:N], d2w[:, 512:N],
                                     G[:, 512:N])

            def L1_group(g):
                c0 = g * N
                tri2(G, xpk, c0)
                ents2(G, xpk, c0)
                nc.vector.tensor_mul(Gp[:, c0:c0 + N], d2w[:, :],
                                     G[:, c0:c0 + N])

            def f1(i, fill=2, fill_mid=2):
                g, j = divmod(i, 4)
                T = big.tile([128, 2 * N], F32, tag="big", name=f"t1_{i}")
                fillers(T, fill)
                # psum chunks must not cross 512-f32 bank boundaries:
                # kh0 occupies tile cols 0:768 (chunks 512+256), kh1 cols
                # 768:1536 (chunks 256+512).
                chunks = (((0, 512), (512, 256)), ((0, 256), (256, 512)))
                for kh in range(2):
                    if kh == 1:
                        fillers(T, fill_mid, c0=N + 256)
                    for cs, w in chunks[kh]:
                        nc.tensor.matmul(
                            T[:, kh * N + cs:kh * N + cs + w],
                            w1[32 * j:32 * j + 6, kh * 128:(kh + 1) * 128],
                            Gp[32 * j:32 * j + 6, g * N + cs:g * N + cs + w],
                            start=True, stop=True, tile_position=(32 * j, 0))
                return T

            def f1_evac(i, T):
                for kh in range(2):
                    dst = h1[:, kh * IPC * N + i * N:(kh * IPC + i + 1) * N]
                    src = T[:, kh * N:(kh + 1) * N]
                    if i in dve_h1 and kh == 1:
                        nc.vector.tensor_scalar(dst, src, 0.0, None, op0=MAX)
                    else:
                        nc.scalar.activation(dst, src, RELU)

            def f2(q, fill=3, fill_mid=1):
                i = 2 * q
                P = big.tile([128, 2 * N], F32, tag="big", name=f"t2_{q}")
                fillers(P, fill)
                for c in (0, 512, 1024):
                    if c and fill_mid:
                        fillers(P, fill_mid, c0=c)
                    for kh in range(2):
                        nc.tensor.matmul(
                            P[:, c:c + 512],
                            w2pc[:, kh * 128:(kh + 1) * 128],
                            h1[:, kh * IPC * N + i * N + c:
                               kh * IPC * N + i * N + c + 512],
                            start=(kh == 0), stop=(kh == 1))
                return P

            def u2_evac(q, P):
                i = 2 * q
                if q == 3:
                    # last pair is tail-critical: halve evac latency by
                    # splitting across ACT and DVE
                    nc.scalar.copy(u2[:, i * N:(i + 1) * N], P[:, 0:N])
                    nc.vector.tensor_copy(u2[:, (i + 1) * N:(i + 2) * N],
                                          P[:, N:2 * N])
                else:
                    nc.scalar.copy(u2[:, i * N:(i + 2) * N], P[:, 0:2 * N])

            def agg2(q):
                i = 2 * q
                tri_pair(S2v, U2v, i, i + 2)
                ents_pair(S2v, U2v, i, i + 2)
                sl = s2[:, i * N:(i + 2) * N]
                nc.vector.tensor_scalar(sl, sl, 0.0, None, op0=MAX)

            def f4(i, Q):
                g, j = divmod(i, 4)
                for cs, w in ((0, 512), (512, 256)):
                    nc.tensor.matmul(
                        Q[32 * j:32 * j + 32, cs:cs + w],
                        w2pc[:, 256:288],
                        s2[:, i * N + cs:i * N + cs + w],
                        start=True, stop=True, tile_position=(0, 32 * j))

            def u3_evac(g, Q):
                nc.vector.tensor_mul(u3g[g][:, :], d2w[:, :], Q[:, 0:N])

            def tail_S3(g):
                tri2(s3g[g], u3g[g], 0)
                ents2(s3g[g], u3g[g], 0)

            def tail_m4(g):
                nc.vector.tensor_mul(m4g[g][:, :], d2w[:, :], s3g[g][:, :])

            def tail_S4(g):
                tri2(s4g[g], m4g[g], 0)
                ents2(s4g[g], m4g[g], 0)

            def out_dma(g):
                nc.sync.dma_start(outq_d[g], s4g[g][:, :])

            # ---- schedule ----------------------------------------------
            # ACT stream keeps each pair's u2 evac AFTER the next pair's
            # first f1 evacs so f2(q+1) is never blocked behind u2e(q);
            # group-0 tail ops are slotted into DVE's between-pair gaps.
            L1_group0_chunked()
            T0 = f1(0)
            f1_evac(0, T0)
            T1 = f1(1)
            L1_group(1)
            nc.vector.tensor_copy(wsink[:, :], wps[:, 0:1])
            f1_evac(1, T1)
            P0 = f2(0)
            T2 = f1(2)
            f1_evac(2, T2)
            u2_evac(0, P0)
            T3 = f1(3)
            f1_evac(3, T3)
            agg2(0)
            P1 = f2(1)
            Qg0 = p4.tile([128, N], F32, tag="p4", name="q0")
            T4 = f1(4)
            f1_evac(4, T4)
            u2_evac(1, P1)
            f4(0, Qg0)
            f4(1, Qg0)
            T5 = f1(5)
            f1_evac(5, T5)
            agg2(1)
            f4(2, Qg0)
            f4(3, Qg0)
            u3_evac(0, Qg0)
            P2 = f2(2)
            T6 = f1(6)
            f1_evac(6, T6)
            u2_evac(2, P2)
            tail_S3(0)
            tail_m4(0)
            T7 = f1(7)
            f1_evac(7, T7)
            agg2(2)
            tail_S4(0)
            out_dma(0)
            P3 = f2(3)
            u2_evac(3, P3)
            agg2(3)
            # g1's f4 psum comes from the big pool (frees on T7's evac) so
            # its fillers can bridge the agg2(3) wait without a coarse
            # p4-pool dependency
            Qg1 = big.tile([128, 2 * N], F32, tag="big", name="q1")
            fillers(Qg1, 8)
            for i in range(4, 8):
                f4(i, Qg1)
            u3_evac(1, Qg1)
            tail_S3(1)
            tail_m4(1)
            tail_S4(1)
            out_dma(1)

            if debug:
                nc.sync.dma_start(dbg_d["gp"][:, :], Gp[:, :])
                nc.sync.dma_start(dbg_d["h1"][:, :], h1[:, :])
                nc.sync.dma_start(dbg_d["u2"][:, :], u2[:, :])
                nc.sync.dma_start(dbg_d["s2"][:, :], s2[:, :])
                for g in range(2):
                    nc.sync.dma_start(dbg_d["u3"][:, g * N:(g + 1) * N],
                                      u3g[g][:, :])
                    nc.sync.dma_start(dbg_d["s3"][:, g * N:(g + 1) * N],
                                      s3g[g][:, :])
                    nc.sync.dma_start(dbg_d["m4"][:, g * N:(g + 1) * N],
                                      m4g[g][:, :])

    return nc


# ---------------------------------------------------------------------------
# v1 dense fallback (bias / unexpected adjacency)
# ---------------------------------------------------------------------------

def _build_program_v1(with_bias: bool):
    nc = bass.Bass()
    KT = N // 128

    xT_d = nc.declare_dram_parameter("xT", [IPC, 6, N], F32, isOutput=False)
    anT_d = nc.declare_dram_parameter("anT", [N, N], F32, isOutput=False)
    a2T_d = nc.declare_dram_parameter("a2T", [N, N], F32, isOutput=False)
    w1T_d = nc.declare_dram_parameter("w1T", [6, 256], F32, isOutput=False)
    w2Tp_d = nc.declare_dram_parameter("w2Tp", [128, 256], F32, isOutput=False)
    w34T_d = nc.declare_dram_parameter("w34T", [128, 3], F32, isOutput=False)
    if with_bias:
        p1t_d = nc.declare_dram_parameter("p1t", [128, 2 * N], F32, isOutput=False)
        p2t_d = nc.declare_dram_parameter("p2t", [128, N], F32, isOutput=False)
        cpt_d = nc.declare_dram_parameter("cpt", [3 * IPC, N], F32, isOutput=False)
    out_d = nc.declare_dram_parameter("outp", [3 * IPC, N], F32, isOutput=True)

    with TileContext(nc) as tc:
        with (
            tc.tile_pool(name="const", bufs=1) as cpool,
            tc.tile_pool(name="acts", bufs=2) as apool,
            tc.tile_pool(name="psf", bufs=2, space="PSUM") as psf,
            tc.tile_pool(name="psa", bufs=3, space="PSUM") as psa,
        ):
            anT = cpool.tile([128, KT * N], F32)
            nc.sync.dma_start(
                anT[:, :].rearrange("p (k j) -> p k j", j=N),
                anT_d[:, :].rearrange("(k p) j -> p k j", p=128))
            a2T = cpool.tile([128, KT * N], F32)
            nc.sync.dma_start(
                a2T[:, :].rearrange("p (k j) -> p k j", j=N),
                a2T_d[:, :].rearrange("(k p) j -> p k j", p=128))
            w1T = cpool.tile([6, 256], F32)
            nc.sync.dma_start(w1T[:, :], w1T_d[:, :])
            w2Tp = cpool.tile([128, 256], F32)
            nc.sync.dma_start(w2Tp[:, :], w2Tp_d[:, :])
            w34T = cpool.tile([128, 3], F32)
            nc.sync.dma_start(w34T[:, :], w34T_d[:, :])
            if with_bias:
                p1t = cpool.tile([128, 2 * N], F32)
                nc.sync.dma_start(p1t[:, :], p1t_d[:, :])
                p2t = cpool.tile([128, N], F32)
                nc.sync.dma_start(p2t[:, :], p2t_d[:, :])
                cpt = cpool.tile([3 * IPC, N], F32)
                nc.sync.dma_start(cpt[:, :], cpt_d[:, :])

            z34 = cpool.tile([128, KT * 3 * IPC], F32)

            for it in range(IPC):
                xT = apool.tile([6, N], F32, tag="xT")
                nc.sync.dma_start(xT[:, :], xT_d[it])

                z1 = apool.tile([128, KT * 256], F32, tag="z1")
                for m in range(KT):
                    ps = psf.tile([128, 256], F32, tag="feat")
                    nc.tensor.matmul(
                        ps[:, :], xT[:, m * 128:(m + 1) * 128], w1T[:, :],
                        start=True, stop=True,
                    )
                    nc.vector.tensor_copy(z1[:, m * 256:(m + 1) * 256], ps[:, :])

                h1t = apool.tile([128, 2 * N], F32, tag="h1t")
                for fh in range(2):
                    for ns in range(2):
                        ps = psa.tile([128, 384], F32, tag="agg")
                        for k in range(KT):
                            nc.tensor.matmul(
                                ps[:, :],
                                z1[:, k * 256 + fh * 128: k * 256 + fh * 128 + 128],
                                anT[:, k * N + ns * 384: k * N + ns * 384 + 384],
                                start=(k == 0), stop=(k == KT - 1),
                            )
                        dst = h1t[:, fh * N + ns * 384: fh * N + ns * 384 + 384]
                        if with_bias:
                            nc.vector.tensor_tensor(
                                dst, ps[:, :],
                                p1t[:, fh * N + ns * 384: fh * N + ns * 384 + 384],
                                op=ADD,
                            )
                            nc.scalar.activation(dst, dst, RELU)
                        else:
                            nc.scalar.activation(dst, ps[:, :], RELU)

                z2 = apool.tile([128, KT * 128], F32, tag="z2")
                for m in range(KT):
                    ps = psf.tile([128, 128], F32, tag="feat")
                    for kh in range(2):
                        nc.tensor.matmul(
                            ps[:, :],
                            h1t[:, kh * N + m * 128: kh * N + m * 128 + 128],
                            w2Tp[:, kh * 128:(kh + 1) * 128],
                            start=(kh == 0), stop=(kh == 1),
                        )
                    nc.vector.tensor_copy(z2[:, m * 128:(m + 1) * 128], ps[:, :])

                h2t = apool.tile([128, N], F32, tag="h2t")
                for ns in range(2):
                    ps = psa.tile([128, 384], F32, tag="agg")
                    for k in range(KT):
                        nc.tensor.matmul(
                            ps[:, :],
                            z2[:, k * 128:(k + 1) * 128],
                            anT[:, k * N + ns * 384: k * N + ns * 384 + 384],
                            start=(k == 0), stop=(k == KT - 1),
                        )
                    dst = h2t[:, ns * 384: ns * 384 + 384]
                    if with_bias:
                        nc.vector.tensor_tensor(
                            dst, ps[:, :], p2t[:, ns * 384: ns * 384 + 384],
                            op=ADD,
                        )
                        nc.scalar.activation(dst, dst, RELU)
                    else:
                        nc.scalar.activation(dst, ps[:, :], RELU)

                for m in range(KT):
                    ps = psf.tile([128, 3], F32, tag="feat")
                    nc.tensor.matmul(
                        ps[:, :], h2t[:, m * 128:(m + 1) * 128], w34T[:, :],
                        start=True, stop=True,
                    )
                    base = m * 3 * IPC + it * 3
                    nc.vector.tensor_copy(z34[:, base: base + 3], ps[:, :])

            outT = cpool.tile([3 * IPC, N], F32)
            for ns in range(2):
                ps = psa.tile([3 * IPC, 384], F32, tag="agg")
                for k in range(KT):
                    nc.tensor.matmul(
                        ps[:, :],
                        z34[:, k * 3 * IPC:(k + 1) * 3 * IPC],
                        a2T[:, k * N + ns * 384: k * N + ns * 384 + 384],
                        start=(k == 0), stop=(k == KT - 1),
                    )
                dst = outT[:, ns * 384: ns * 384 + 384]
                if with_bias:
                    nc.vector.tensor_tensor(
                        dst, ps[:, :], cpt[:, ns * 384: ns * 384 + 384],
                        op=ADD,
                    )
                else:
                    nc.vector.tensor_copy(dst, ps[:, :])
            nc.sync.dma_start(out_d[:, :], outT[:, :])

    return nc


def kernel(x, inputs, adjacency, W1, b1, W2, b2, W3, b3, W4, b4,
           parent_sel, child1_sel, child2_sel):
    global LAST_RUN_INFO
    x = np.asarray(x, np.float32)
    inp = np.asarray(inputs, np.float32)
    A = np.asarray(adjacency, np.float32)
    W1 = np.asarray(W1, np.float32); b1 = np.asarray(b1, np.float32)
    W2 = np.asarray(W2, np.float32); b2 = np.asarray(b2, np.float32)
    W3 = np.asarray(W3, np.float32); b3 = np.asarray(b3, np.float32)
    W4 = np.asarray(W4, np.float32); b4 = np.asarray(b4, np.float32)
    parent_sel = np.asarray(parent_sel, np.int64)
    child1_sel = np.asarray(child1_sel, np.int64)
    child2_sel = np.asarray(child2_sel, np.int64)

    clamp_rows = np.concatenate([
        parent_sel, NV + child1_sel, 2 * NV + child2_sel,
    ]).astype(np.int64)

    x0 = x.copy()
    x0[:, clamp_rows, 0:3] = inp[:, clamp_rows, :]

    deg = A.sum(axis=-1)
    deg_safe = np.where(deg == 0, np.float32(1.0), deg)
    d = np.where(deg == 0, np.float32(0.0),
                 deg_safe ** np.float32(-0.5)).astype(np.float32)
    A_norm = (A * d[:, None] * d[None, :]).astype(np.float32)

    with_bias = bool(np.any(b1) or np.any(b2) or np.any(b3) or np.any(b4))
    use_v4 = ((not with_bias) and _structure_matches(A_norm, d)
              and _d2_pattern_matches(d))

    trace = os.environ.get("KERNEL_TRACE", "") == "1"

    if use_v4:
        bf = ml_dtypes.bfloat16
        Xd = (d[None, :, None] * x0).astype(bf)                   # (B, N, 6)
        # xq[core, 32j+f, g*N + n] = Xd[core*8 + g*4 + j, n, f]
        Xr = Xd.reshape(NCORES, 2, 4, N, 6).transpose(0, 2, 4, 1, 3)
        xq = np.zeros((NCORES, 4, 32, 2 * N), bf)
        xq[:, :, 0:6, :] = Xr.reshape(NCORES, 4, 6, 2 * N)
        xq = xq.reshape(NCORES, 128, 2 * N)

        w1q = np.zeros((4, 32, 256), bf)
        w1q[:, 0:6, :] = W1.T.astype(bf)[None]
        w1q = np.ascontiguousarray(w1q.reshape(128, 256))
        w2pc = np.zeros((128, 288), bf)
        w2pc[:, 0:256] = np.ascontiguousarray(
            W2.T.reshape(2, 128, 128).transpose(1, 0, 2).reshape(128, 256)
        ).astype(bf)
        w2pc[:, 256:259] = (W3.T @ W4.T).astype(bf)

        nc = _build_program_v4()
        _split_multi_waits(nc)
        in_maps = [{"xq": xq[c], "w1q": w1q, "w2pc": w2pc}
                   for c in range(NCORES)]

        res = run_bass_kernel_spmd(nc, in_maps, list(range(NCORES)),
                                   trace=trace)
        LAST_RUN_INFO = {
            "exec_time_ns": res.exec_time_ns,
            "mean_exec_time_ns": res.mean_exec_time_ns,
            "max_exec_time_core_id": res.max_exec_time_core_id,
        }

        out = np.empty((B, N, 3), np.float32)
        for c in range(NCORES):
            o = np.asarray(res.results[c]["outq"], bf).astype(np.float32)
            # o[g, 32j+s, n] -> out[c*8 + g*4 + j, n, s]
            oi = o.reshape(2, 4, 32, N)[:, :, 0:3, :]     # (g, j, s, n)
            arr = oi.transpose(0, 1, 3, 2).reshape(IPC, N, 3)
            out[c * IPC:(c + 1) * IPC] = arr
        out *= d[None, :, None]
    else:
        AnT = np.ascontiguousarray(A_norm.T)
        A2T = np.ascontiguousarray((A_norm @ A_norm).T.astype(np.float32))
        W1T = np.ascontiguousarray(W1.T)
        W2Tp = np.ascontiguousarray(
            W2.T.reshape(2, 128, 128).transpose(1, 0, 2).reshape(128, 256))
        W34T = np.ascontiguousarray(W3.T @ W4.T)

        extra = {}
        if with_bias:
            s = A_norm.sum(axis=1).astype(np.float32)
            s2 = (A_norm @ s).astype(np.float32)
            p1t = np.einsum('f,n->fn', b1, s).astype(np.float32)
            p1t = p1t.reshape(2, 128, N).transpose(1, 0, 2).reshape(128, 2 * N)
            p2t = np.einsum('f,n->fn', b2, s).astype(np.float32)
            cp = (np.einsum('f,n->fn', W4 @ b3, s2) +
                  np.einsum('f,n->fn', b4, s)).astype(np.float32)
            cpt = np.tile(cp, (IPC, 1)).astype(np.float32)
            extra = {"p1t": np.ascontiguousarray(p1t),
                     "p2t": np.ascontiguousarray(p2t),
                     "cpt": np.ascontiguousarray(cpt)}

        xT_all = np.ascontiguousarray(
            x0.transpose(0, 2, 1).reshape(NCORES, IPC, 6, N))

        nc = _build_program_v1(with_bias)
        _split_multi_waits(nc)

        in_maps = []
        for c in range(NCORES):
            m = {
                "xT": xT_all[c], "anT": AnT, "a2T": A2T,
                "w1T": W1T, "w2Tp": W2Tp, "w34T": W34T,
            }
            m.update(extra)
            in_maps.append(m)

        res = run_bass_kernel_spmd(nc, in_maps, list(range(NCORES)),
                                   trace=trace)
        LAST_RUN_INFO = {
            "exec_time_ns": res.exec_time_ns,
            "mean_exec_time_ns": res.mean_exec_time_ns,
            "max_exec_time_core_id": res.max_exec_time_core_id,
        }

        out = np.empty((B, N, 3), np.float32)
        for c in range(NCORES):
            o = res.results[c]["outp"]
            for it in range(IPC):
                out[c * IPC + it] = o[it * 3:(it + 1) * 3, :].T
    out[:, clamp_rows, :] = inp[:, clamp_rows, :]
    return out
